# revision 26
# baseline (speedup 1.0000x reference)
"""Trainium2 Bass kernel for 3-layer GAT + global_add_pool + linear head.

Design (v3):
- Nodes (and incoming edges) sharded across 8 cores by dst.
- Node phase per layer: h_aug = x @ [W | W@As | W@Ad] on PE (bf16 in, f32 out).
  [h|alpha_src] rows (bf16, 512B stride) go to a DRAM table that is
  AllGathered across cores.
- Edge phase: edges tiled 128/dst-block (sub-split by src half for int16
  gather indices); per chunk of CT tiles, batched InstDMAGatherAnt fetches
  h[src] rows on 4 SWDGE queues. Per-edge alpha_dst comes from transposed
  selector matmuls on the (otherwise idle) PE; selectors are built by vector
  is_equal against a streamed dst-slot pattern.
- e = lrelu(a_s+a_d) on vector (max(z,.2z)), exp batched on scalar, messages
  scaled in place (bf16), then per-tile selector matmuls accumulate [w*h | w]
  into PSUM per dst block; normalization + relu on block end.
- Partial pooled logits summed on host.

Self-contained: no file reads; shapes hardcoded via constants.
"""
import math
import numpy as np
from contextlib import ExitStack

import concourse.bass as bass
import concourse.mybir as mybir
import concourse.tile as tile
from concourse.bass_utils import run_bass_kernel_spmd
from concourse.tile_rust import add_dep_helper
from concourse import library_config

NCORES = 8
P = 128
H = 4
Ch = 32
HC = 128
AUGW = HC + 2 * H   # 136: node matmul out [h | a_src | a_dst]
TBL = HC + H        # 132: useful row [h | a_src]
TW = 256            # bf16 table row width (512B stride)
NEG_SLOPE = 0.2
GRAPHS = 64
OUT = 10
CT = 32             # tiles (of 128 edges) per chunk
NSWQ = 4            # SWDGE queues
BA = 24             # blocks in table piece A (per core), capped to nb//2


def _ba(nb):
    return max(1, min(BA, nb // 2))
CC_HIDE_CHUNK = 20  # edge-phase chunk index at which next layer's AG-A issues
import os as _os
K_LRELU = _os.environ.get("K_LRELU", "0") == "1"
K_RELU = _os.environ.get("K_RELU", "1") == "1"

# instruction types whose BIR struct cannot carry all Tile-emitted waits
_WAIT_CAPS = {
    "InstDMAGatherAnt": 0,
    "InstDMAScatterAddAnt": 0,
    "InstNoOp": 1,
    "InstDrain": 1,
    "InstCollectiveCompute": 1,
}


def _fixup_wait_limits(nc):
    k = 0
    for fn in nc.m.functions:
        for blk in fn.blocks:
            out = []
            for inst in blk.instructions:
                cap = _WAIT_CAPS.get(type(inst).__name__, 1)
                si = inst.sync_info
                if si is not None:
                    waits = list(si.on_wait)
                    if len(waits) > cap:
                        keep, move = waits[:cap], waits[cap:]
                        for w in move:
                            nop = mybir.InstNoOp(name=f"waitfix_{k}", text_hint="wait_fixup")
                            k += 1
                            nop.engine = inst.engine
                            nop.sync_info = type(si)(on_wait=[w], on_update=[])
                            out.append(nop)
                        inst.sync_info = type(si)(on_wait=list(keep), on_update=list(si.on_update))
                out.append(inst)
            blk.instructions = out
    return k


def _prep_edges(src_all, dst_all, per, nb, npad):
    """Tile edges per core by (dst block, src half), build gather idx arrays.

    Returns (blk_of_tile, start_t, stop_t, bufcol, n_lo, Tpad,
    srcidxs, dlocs, dlocTs): srcidx is the per-core [128, Tpad*8] int16 SBUF
    image; dloc is [P, Tpad] f32 dst-slot per edge slot (buffer order, -1
    pad); dlocT is [1, Tpad*128] bf16 dst-slot along free dim (buffer order).
    """
    NHALF = npad // 2
    percore = []
    cnt_bh = np.zeros((nb, 2), np.int64)
    for c in range(NCORES):
        m = (dst_all // per) == c
        s = src_all[m]
        loc = dst_all[m] - c * per
        b = loc // P
        sc = s // per
        srem = s % per
        hf = (srem // P >= _ba(nb)).astype(np.int64)
        order = np.lexsort((hf, b))
        s, loc, b, hf = s[order], loc[order], b[order], hf[order]
        sc, srem = sc[order], srem[order]
        cnt = np.bincount(b * 2 + hf, minlength=nb * 2).reshape(nb, 2)
        cnt_bh = np.maximum(cnt_bh, cnt)
        ba = _ba(nb)
        rloc = np.where(srem < ba * P, sc * (ba * P) + srem,
                        sc * (per - ba * P) + srem - ba * P)
        percore.append((rloc, loc, b, hf, cnt))

    tiles_bh = (cnt_bh + P - 1) // P            # [nb, 2]
    # processing-order tiles
    blk_l, hf_l = [], []
    tstart = np.zeros((nb, 2), np.int64)
    t = 0
    for b in range(nb):
        for hf in (0, 1):
            tstart[b, hf] = t
            n = int(tiles_bh[b, hf])
            blk_l += [b] * n
            hf_l += [hf] * n
            t += n
    T = t
    nchunks = (T + CT - 1) // CT
    Tpad = nchunks * CT
    blk_of_tile = np.array(blk_l + [nb - 1] * (Tpad - T), np.int64)
    hf_of_tile = np.array(hf_l + [0] * (Tpad - T), np.int64)
    start_t = np.zeros(Tpad, bool)
    stop_t = np.zeros(Tpad, bool)
    for b in range(nb):
        w = np.nonzero(blk_of_tile == b)[0]
        start_t[w[0]] = True
        stop_t[w[-1]] = True

    # buffer-column mapping: per chunk, lo tiles first then hi tiles
    bufcol = np.zeros(Tpad, np.int64)
    n_lo = []
    for ch in range(nchunks):
        ts = np.arange(ch * CT, (ch + 1) * CT)
        lo = ts[hf_of_tile[ts] == 0]
        hi = ts[hf_of_tile[ts] == 1]
        bufcol[lo] = np.arange(len(lo))
        bufcol[hi] = len(lo) + np.arange(len(hi))
        n_lo.append(len(lo))
    g2b = (np.arange(Tpad) // CT) * CT + bufcol   # proc tile -> buffer col

    import ml_dtypes
    SW = Tpad * 8
    srcidxs, dlocs, dlocTs = [], [], []
    for c in range(NCORES):
        rloc, loc, b, hf, cnt = percore[c]
        srcv = np.zeros((P, Tpad), np.int64)
        dlp = np.full((P, Tpad), -1.0, np.float32)   # proc order
        off = np.zeros(nb * 2 + 1, np.int64)
        off[1:] = np.cumsum(cnt.reshape(-1))
        key = b * 2 + hf
        pos = np.arange(len(rloc)) - off[key]
        tt = tstart[b, hf] + pos // P
        pp = pos % P
        srcv[pp, tt] = rloc
        dlp[pp, tt] = loc % P
        # sbuf idx image: slot (p, t) -> row 16k+p%16, col (bufgcol*8)+p//16
        p_g, t_g = np.mgrid[0:P, 0:Tpad]
        col = g2b[t_g] * 8 + p_g // 16
        row = p_g % 16
        si = np.zeros((P, SW), np.int16)
        for k in range(8):
            si[16 * k + row, col] = srcv
        srcidxs.append(si)
        dloc_buf = np.full((P, Tpad), -1.0, np.float32)
        dloc_buf[:, g2b] = dlp
        dlocs.append(dloc_buf.astype(ml_dtypes.bfloat16))
        dT = np.full((Tpad, P), -1.0, np.float32)
        dT[g2b, :] = dlp.T
        dlocTs.append(dT.reshape(1, Tpad * P).astype(ml_dtypes.bfloat16))
    return (blk_of_tile, start_t, stop_t, bufcol, n_lo, Tpad,
            srcidxs, dlocs, dlocTs)


def _build(npad, Tpad, blk_of_tile, start_t, stop_t, bufcol, n_lo):
    per = npad // NCORES
    nb = per // P
    NHALF = npad // 2
    nlayers = 3
    nchunks = Tpad // CT
    f32 = mybir.dt.float32
    bf16 = mybir.dt.bfloat16
    i16 = mybir.dt.int16

    nc = bass.Bass(num_devices=NCORES, num_swdge_queues=NSWQ)
    # ---- dram I/O
    xT_d = nc.dram_tensor("xT", [P, per], bf16, kind="ExternalInput")
    waug_d = nc.dram_tensor("waug", [nlayers, P, AUGW], bf16, kind="ExternalInput")
    wh_d = nc.dram_tensor("wh", [P, OUT], f32, kind="ExternalInput")
    iota_d = nc.dram_tensor("iota", [P, 4 * P], f32, kind="ExternalInput")
    iotap_d = nc.dram_tensor("iotap", [P, 1], bf16, kind="ExternalInput")
    ident_d = nc.dram_tensor("ident", [P, P], f32, kind="ExternalInput")
    SW = Tpad * 8
    srcidx_d = nc.dram_tensor("srcidx", [P, SW], i16, kind="ExternalInput")
    dloc_d = nc.dram_tensor("dloc", [P, Tpad], bf16, kind="ExternalInput")
    iotarep_d = nc.dram_tensor("iotarep", [P, CT * P], bf16, kind="ExternalInput")
    dlocT_d = nc.dram_tensor("dlocT", [1, Tpad * P], bf16, kind="ExternalInput")
    batchf_d = nc.dram_tensor("batchf", [P, nb], f32, kind="ExternalInput")
    out_d = nc.dram_tensor("out", [GRAPHS, OUT], f32, kind="ExternalOutput")

    h_loc = [nc.dram_tensor(f"h_loc{l}", [per, TW], bf16) for l in range(nlayers)]
    h_full = [nc.dram_tensor(f"h_full{l}", [npad, TW], bf16, addr_space="Shared")
              for l in range(nlayers)]

    groups = [list(range(NCORES))]

    with ExitStack() as ctx:
        tc = ctx.enter_context(tile.TileContext(nc))
        sb = ctx.enter_context(tc.tile_pool(name="sb", bufs=1))
        sb_g = ctx.enter_context(tc.tile_pool(name="sbg", bufs=3))
        sb_s = ctx.enter_context(tc.tile_pool(name="sbs", bufs=3))
        sb_t = ctx.enter_context(tc.tile_pool(name="sbt", bufs=3))
        sb_w = ctx.enter_context(tc.tile_pool(name="sbw", bufs=3))
        ps_h = ctx.enter_context(tc.tile_pool(name="psh", bufs=2, space="PSUM"))
        ps_agg = ctx.enter_context(tc.tile_pool(name="psagg", bufs=2, space="PSUM"))
        ps_ad = ctx.enter_context(tc.tile_pool(name="psad", bufs=2, space="PSUM"))
        ps_xp = ctx.enter_context(tc.tile_pool(name="psxp", bufs=1, space="PSUM"))
        ps_fin = ctx.enter_context(tc.tile_pool(name="psfin", bufs=1, space="PSUM"))

        # ---- persistent SBUF state
        xT = sb.tile([P, per], bf16)
        nc.sync.dma_start(out=xT[:], in_=xT_d[:])
        waug = sb.tile([P, nlayers, AUGW], bf16)
        nc.sync.dma_start(out=waug[:],
                          in_=waug_d[:].rearrange("l p a -> p l a"))
        wh = sb.tile([P, OUT], f32)
        nc.sync.dma_start(out=wh[:], in_=wh_d[:])
        iota = sb.tile([P, 4, P], f32)
        nc.sync.dma_start(out=iota[:].rearrange("p a b -> p (a b)"), in_=iota_d[:])
        iotap = sb.tile([P, 1], bf16)
        nc.sync.dma_start(out=iotap[:], in_=iotap_d[:])
        srci = sb.tile([P, SW], i16)
        nc.sync.dma_start(out=srci[:], in_=srcidx_d[:])
        dloc = sb.tile([P, Tpad, 1], bf16)
        nc.sync.dma_start(out=dloc[:].rearrange("p t o -> p (t o)"), in_=dloc_d[:])
        iotarep = sb.tile([P, CT, P], bf16)
        nc.sync.dma_start(out=iotarep[:].rearrange("p c e -> p (c e)"),
                          in_=iotarep_d[:])
        batchf = sb.tile([P, nb, 1], f32)
        nc.sync.dma_start(out=batchf[:].rearrange("p b o -> p (b o)"), in_=batchf_d[:])
        identf = sb.tile([P, P], f32)
        nc.sync.dma_start(out=identf[:], in_=ident_d[:])

        nc.gpsimd.load_library(library_config.mlp)

        _regs = {}

        def nreg(v):
            if v not in _regs:
                _regs[v] = nc.gpsimd.to_reg(v)
            return _regs[v]

        hsb2 = [sb.tile([P, nb, TBL], bf16, name=f"hsb{i}") for i in range(2)]
        adsb2 = [sb.tile([P, nb, H], bf16, name=f"adsb{i}") for i in range(2)]
        pooled_ps = ps_fin.tile([GRAPHS, HC], f32)
        qn = [0]

        def nextq():
            qn[0] = (qn[0] + 1) % NSWQ
            return qn[0]

        ba = _ba(nb)
        RA = ba * P            # piece-A local rows
        GA = NCORES * RA       # piece-A global rows
        dhA = [None] * 3
        dhB = [None] * 3
        ccA = [None] * 3
        ccB = [None] * 3

        def node_mm(l, b):
            hsb, adsb = hsb2[l % 2], adsb2[l % 2]
            ps = ps_h.tile([P, AUGW], f32, tag="ndps")
            nc.tensor.matmul(ps[:], lhsT=xT[:, b * P:(b + 1) * P],
                             rhs=waug[:, l, :], start=True, stop=True)
            nc.vector.tensor_copy(out=hsb[:, b, :], in_=ps[:, :TBL])
            nc.vector.tensor_copy(out=adsb[:, b, :], in_=ps[:, TBL:AUGW])
            if b == ba - 1:
                dhA[l] = nc.sync.dma_start(
                    out=h_loc[l][0:RA, 0:TBL].rearrange("(b p) d -> p b d", p=P),
                    in_=hsb[:, 0:ba, :])
            if b == nb - 1:
                dhB[l] = nc.sync.dma_start(
                    out=h_loc[l][RA:per, 0:TBL].rearrange("(b p) d -> p b d", p=P),
                    in_=hsb[:, ba:nb, :])

        def emit_ccA(l):
            ccA[l] = nc.gpsimd.collective_compute(
                "AllGather", mybir.AluOpType.bypass, replica_groups=groups,
                ins=[h_loc[l][0:RA, :]], outs=[h_full[l][0:GA, :]])
            add_dep_helper(ccA[l].ins, dhA[l].ins, sync=True, reason="hA before ag")

        def emit_ccB(l):
            ccB[l] = nc.gpsimd.collective_compute(
                "AllGather", mybir.AluOpType.bypass, replica_groups=groups,
                ins=[h_loc[l][RA:per, :]], outs=[h_full[l][GA:npad, :]])
            add_dep_helper(ccB[l].ins, dhB[l].ins, sync=True, reason="hB before ag")

        # layer-0 node phase prologue
        for b in range(nb):
            node_mm(0, b)
        emit_ccA(0)
        emit_ccB(0)

        for l in range(3):
            adsb = adsb2[l % 2]
            # ===== edge phase =====
            agg_of_blk = {}
            for ci in range(nchunks):
                t0 = ci * CT
                nlo = int(n_lo[ci])
                if (l < 2 and ccA[l + 1] is None and dhA[l + 1] is not None
                        and ci >= CC_HIDE_CHUNK):
                    emit_ccA(l + 1)
                g = sb_g.tile([P, CT, TW], bf16, tag="gath")
                if nlo > 0:
                    glo = nc.gpsimd.dma_gather(
                        out_ap=g[:, 0:nlo, :], in_ap=h_full[l][0:GA, :],
                        idxs_ap=srci[:, t0 * 8:t0 * 8 + nlo * 8],
                        num_idxs=nlo * P, num_idxs_reg=nreg(nlo * P), elem_size=TW,
                        single_packet=False, queue_num=nextq())
                    add_dep_helper(glo.ins, ccA[l].ins, sync=True, reason="gather after agA")
                if nlo < CT:
                    ghi = nc.gpsimd.dma_gather(
                        out_ap=g[:, nlo:CT, :], in_ap=h_full[l][GA:npad, :],
                        idxs_ap=srci[:, t0 * 8 + nlo * 8:(t0 + CT) * 8],
                        num_idxs=(CT - nlo) * P, num_idxs_reg=nreg((CT - nlo) * P),
                        elem_size=TW, single_packet=False, queue_num=nextq())
                    add_dep_helper(ghi.ins, ccB[l].ins, sync=True, reason="gather after agB")

                # transposed selectors (node-slot one-hot along partitions)
                dT = sb_t.tile([P, CT, P], bf16, tag="dT")
                nc.sync.dma_start(
                    out=dT[:].rearrange("p c e -> p (c e)"),
                    in_=dlocT_d[0:1, t0 * P:(t0 + CT) * P].to_broadcast(
                        [P, CT * P]))
                selT = sb_t.tile([P, CT, P], bf16, tag="selT")
                nc.vector.tensor_tensor(
                    out=selT[:], in0=dT[:],
                    in1=iotap[:].rearrange("p (c e) -> p c e", c=1).to_broadcast(
                        [P, CT, P]),
                    op=mybir.AluOpType.is_equal)
                # per-edge alpha_dst via PE: adps[:, j, :] = selT_j^T @ adsb[b]
                adps = ps_ad.tile([P, CT, H], f32, tag="adps")
                for t in range(t0, t0 + CT):
                    b = int(blk_of_tile[t])
                    j = int(bufcol[t])
                    nc.tensor.matmul(adps[:, j, :], lhsT=selT[:, j, :],
                                     rhs=adsb[:, b, :], start=True, stop=True)

                # e = lrelu(a_s + a_d); w = exp(e) written into g cols HC:TBL
                lg = sb_w.tile([P, CT, H], f32, tag="lg")
                nc.vector.tensor_tensor(out=lg[:], in0=adps[:],
                                        in1=g[:, :, HC:TBL],
                                        op=mybir.AluOpType.add)
                lr = sb_w.tile([P, CT, H], f32, tag="lr")
                if K_LRELU:
                    nc.scalar.activation(lr[:], lg[:],
                                         mybir.ActivationFunctionType.Lrelu,
                                         alpha=NEG_SLOPE)
                else:
                    nc.vector.tensor_scalar_mul(lr[:], lg[:], NEG_SLOPE)
                    nc.vector.tensor_tensor(out=lr[:], in0=lr[:], in1=lg[:],
                                            op=mybir.AluOpType.max)
                nc.scalar.activation(g[:, :, HC:TBL], lr[:],
                                     mybir.ActivationFunctionType.Exp)
                # msg in place: g[:, :, h*Ch:(h+1)*Ch] *= w[h]
                nc.vector.tensor_tensor(
                    out=g[:, :, 0:HC].rearrange("p c (h w) -> p c h w", h=H),
                    in0=g[:, :, 0:HC].rearrange("p c (h w) -> p c h w", h=H),
                    in1=g[:, :, HC:TBL].rearrange("p c (h o) -> p c h o", o=1)
                        .to_broadcast([P, CT, H, Ch]),
                    op=mybir.AluOpType.mult)
                # selectors for the whole chunk (dst one-hot per edge)
                sel = sb_s.tile([P, CT, P], bf16, tag="sel")
                nc.vector.tensor_tensor(
                    out=sel[:], in0=iotarep[:],
                    in1=dloc[:, t0:t0 + CT, :].to_broadcast([P, CT, P]),
                    op=mybir.AluOpType.is_equal)
                # aggregate per tile (processing order)
                for t in range(t0, t0 + CT):
                    b = int(blk_of_tile[t])
                    j = int(bufcol[t])
                    if start_t[t]:
                        agg_of_blk[b] = ps_agg.tile([P, TBL], f32, tag="agg",
                                                    name=f"agg{l}_{b}")
                    nc.tensor.matmul(agg_of_blk[b][:], lhsT=sel[:, j, :],
                                     rhs=g[:, j, 0:TBL],
                                     start=bool(start_t[t]),
                                     stop=bool(stop_t[t]))
                    if stop_t[t]:
                        agg = agg_of_blk.pop(b)
                        rec = sb_w.tile([P, H], f32, tag="rec")
                        nc.vector.reciprocal(rec[:], agg[:, HC:TBL])
                        xb = sb_w.tile([P, HC], f32, tag="xb")
                        nc.vector.tensor_tensor(
                            out=xb[:].rearrange("p (h w) -> p h w", h=H),
                            in0=agg[:, 0:HC].rearrange("p (h w) -> p h w", h=H),
                            in1=rec[:].rearrange("p (h o) -> p h o", o=1)
                                .to_broadcast([P, H, Ch]),
                            op=mybir.AluOpType.mult)
                        if K_RELU:
                            nc.scalar.activation(xb[:], xb[:],
                                                 mybir.ActivationFunctionType.Relu)
                        else:
                            nc.vector.tensor_scalar_max(xb[:], xb[:], 0.0)
                        if l < 2:
                            xps = ps_xp.tile([P, P], f32, tag="xps")
                            nc.tensor.transpose(xps[:], xb[:], identf[:])
                            nc.vector.tensor_copy(
                                out=xT[:, b * P:(b + 1) * P], in_=xps[:])
                            node_mm(l + 1, b)
                        else:
                            bsel = sb_w.tile([P, GRAPHS], f32, tag="bsel")
                            nc.vector.tensor_tensor(
                                out=bsel[:],
                                in0=batchf[:, b, :].to_broadcast([P, GRAPHS]),
                                in1=iota[:, 0, :GRAPHS],
                                op=mybir.AluOpType.is_equal)
                            nc.tensor.matmul(pooled_ps[:], lhsT=bsel[:],
                                             rhs=xb[:], start=(b == 0),
                                             stop=(b == nb - 1))
            if l < 2:
                if ccA[l + 1] is None:
                    emit_ccA(l + 1)
                emit_ccB(l + 1)

        # ===== head =====
        pooled_sb = sb.tile([GRAPHS, HC], f32)
        nc.vector.tensor_copy(out=pooled_sb[:], in_=pooled_ps[:])
        pT_ps = ps_xp.tile([P, GRAPHS], f32, tag="xps")
        nc.tensor.transpose(pT_ps[:], pooled_sb[:], identf[:GRAPHS, :GRAPHS])
        pT_sb = sb.tile([P, GRAPHS], f32)
        nc.vector.tensor_copy(out=pT_sb[:], in_=pT_ps[:])
        log_ps = ps_xp.tile([GRAPHS, OUT], f32, tag="xps")
        nc.tensor.matmul(log_ps[:], lhsT=pT_sb[:], rhs=wh[:], start=True, stop=True)
        log_sb = sb.tile([GRAPHS, OUT], f32)
        nc.vector.tensor_copy(out=log_sb[:], in_=log_ps[:])
        nc.sync.dma_start(out=out_d[:], in_=log_sb[:])

    _fixup_wait_limits(nc)
    mybir.codegen_inst_isa_subclasses(nc)
    return nc


def prepare(x, Ws, a_srcs, a_dsts, biases, Wh, bh, edge_index, batch):
    n = x.shape[0]
    npad = int(math.ceil(n / (NCORES * P)) * NCORES * P)
    per = npad // NCORES
    nb = per // P

    x = np.asarray(x, np.float32)
    Ws = [np.asarray(w, np.float32) for w in Ws]
    a_srcs = [np.asarray(a, np.float32) for a in a_srcs]
    a_dsts = [np.asarray(a, np.float32) for a in a_dsts]
    Wh = np.asarray(Wh, np.float32)
    bh = np.asarray(bh, np.float32)
    edge_index = np.asarray(edge_index)
    batch = np.asarray(batch)
    for b in biases:
        assert np.allclose(np.asarray(b), 0.0), "nonzero GAT biases unsupported"

    import ml_dtypes
    # W_aug = [W | W@As | W@Ad]
    waugs = []
    for l in range(3):
        As = np.zeros((HC, H), np.float32)
        Ad = np.zeros((HC, H), np.float32)
        for h in range(H):
            As[h * Ch:(h + 1) * Ch, h] = a_srcs[l][h]
            Ad[h * Ch:(h + 1) * Ch, h] = a_dsts[l][h]
        W = Ws[l]
        waugs.append(np.concatenate([W, W @ As, W @ Ad], axis=1))
    waug = np.stack(waugs, 0).astype(ml_dtypes.bfloat16)  # [3, 128, AUGW]

    # edges + self loops (incl. pad nodes, so every row has >=1 edge)
    src_all = np.concatenate([edge_index[0].astype(np.int64),
                              np.arange(npad, dtype=np.int64)])
    dst_all = np.concatenate([edge_index[1].astype(np.int64),
                              np.arange(npad, dtype=np.int64)])
    (blk_of_tile, start_t, stop_t, bufcol, n_lo, Tpad,
     srcidxs, dlocs, dlocTs) = _prep_edges(src_all, dst_all, per, nb, npad)

    xpad = np.zeros((npad, HC), np.float32)
    xpad[:n] = x
    iota = np.tile(np.arange(P, dtype=np.float32)[None, :], (P, 4))

    batchf_full = np.full(npad, -1.0, np.float32)
    batchf_full[:n] = batch.astype(np.float32)

    nc = _build(npad, Tpad, blk_of_tile, start_t, stop_t, bufcol, n_lo)

    in_maps = []
    for c in range(NCORES):
        sl = slice(c * per, (c + 1) * per)
        in_maps.append({
            "xT": np.ascontiguousarray(xpad[sl].T).astype(ml_dtypes.bfloat16),
            "waug": waug,
            "wh": Wh,
            "iota": iota,
            "iotap": np.arange(P, dtype=np.float32)[:, None].astype(
                ml_dtypes.bfloat16),
            "ident": np.eye(P, dtype=np.float32),
            "srcidx": srcidxs[c],
            "dloc": dlocs[c],
            "iotarep": np.tile(np.arange(P, dtype=np.float32)[None, :],
                               (P, CT)).astype(ml_dtypes.bfloat16),
            "dlocT": dlocTs[c],
            "batchf": np.ascontiguousarray(
                batchf_full[sl].reshape(nb, P).T),
            })
    return nc, in_maps


def run_gat(x, Ws, a_srcs, a_dsts, biases, Wh, bh, edge_index, batch):
    nc, in_maps = prepare(x, Ws, a_srcs, a_dsts, biases, Wh, bh,
                          edge_index, batch)
    res = run_bass_kernel_spmd(nc, in_maps, list(range(NCORES)))
    global LAST_EXEC_NS
    LAST_EXEC_NS = getattr(res, "exec_time_ns", None)
    logits = np.zeros((GRAPHS, OUT), np.float32)
    for c in range(NCORES):
        logits += res.results[c]["out"]
    return logits + bh


def kernel(**inputs):
    return np.asarray(run_gat(
        inputs["x"], inputs["Ws"], inputs["a_srcs"], inputs["a_dsts"],
        inputs["biases"], inputs["Wh"], inputs["bh"], inputs["edge_index"],
        inputs["batch"]), np.float32)


# revision 27
# speedup vs baseline: 1.0564x; 1.0564x over previous
"""Trainium2 Bass kernel for 3-layer GAT + global_add_pool + linear head.

Design (v3):
- Nodes (and incoming edges) sharded across 8 cores by dst.
- Node phase per layer: h_aug = x @ [W | W@As | W@Ad] on PE (bf16 in, f32 out).
  [h|alpha_src] rows (bf16, 512B stride) go to a DRAM table that is
  AllGathered across cores.
- Edge phase: edges tiled 128/dst-block (sub-split by src half for int16
  gather indices); per chunk of CT tiles, batched InstDMAGatherAnt fetches
  h[src] rows on 4 SWDGE queues. Per-edge alpha_dst comes from transposed
  selector matmuls on the (otherwise idle) PE; selectors are built by vector
  is_equal against a streamed dst-slot pattern.
- e = lrelu(a_s+a_d) on vector (max(z,.2z)), exp batched on scalar, messages
  scaled in place (bf16), then per-tile selector matmuls accumulate [w*h | w]
  into PSUM per dst block; normalization + relu on block end.
- Partial pooled logits summed on host.

Self-contained: no file reads; shapes hardcoded via constants.
"""
import math
import numpy as np
from contextlib import ExitStack

import concourse.bass as bass
import concourse.mybir as mybir
import concourse.tile as tile
from concourse.bass_utils import run_bass_kernel_spmd
from concourse.tile_rust import add_dep_helper
from concourse import library_config

NCORES = 8
P = 128
H = 4
Ch = 32
HC = 128
AUGW = HC + 2 * H   # 136: node matmul out [h | a_src | a_dst]
TBL = HC + H        # 132: useful row [h | a_src]
TW = 256            # bf16 table row width (512B stride)
NEG_SLOPE = 0.2
GRAPHS = 64
OUT = 10
CT = 32             # tiles (of 128 edges) per chunk
NSWQ = 4            # SWDGE queues
BA = 24             # blocks in table piece A (per core), capped to nb//2


def _ba(nb):
    return max(1, min(BA, nb // 2))
CC_HIDE_CHUNK = 20  # edge-phase chunk index at which next layer's AG-A issues
import os as _os
K_LRELU = _os.environ.get("K_LRELU", "0") == "1"
K_RELU = _os.environ.get("K_RELU", "1") == "1"

# instruction types whose BIR struct cannot carry all Tile-emitted waits
_WAIT_CAPS = {
    "InstDMAGatherAnt": 0,
    "InstDMAScatterAddAnt": 0,
    "InstNoOp": 1,
    "InstDrain": 1,
    "InstCollectiveCompute": 1,
}


def _fixup_wait_limits(nc):
    k = 0
    for fn in nc.m.functions:
        for blk in fn.blocks:
            out = []
            for inst in blk.instructions:
                cap = _WAIT_CAPS.get(type(inst).__name__, 1)
                si = inst.sync_info
                if si is not None:
                    waits = list(si.on_wait)
                    if len(waits) > cap:
                        keep, move = waits[:cap], waits[cap:]
                        for w in move:
                            nop = mybir.InstNoOp(name=f"waitfix_{k}", text_hint="wait_fixup")
                            k += 1
                            nop.engine = inst.engine
                            nop.sync_info = type(si)(on_wait=[w], on_update=[])
                            out.append(nop)
                        inst.sync_info = type(si)(on_wait=list(keep), on_update=list(si.on_update))
                out.append(inst)
            blk.instructions = out
    return k


def _prep_edges(src_all, dst_all, per, nb, npad):
    """Tile edges per core by (dst block, src half), build gather idx arrays.

    Returns (blk_of_tile, start_t, stop_t, bufcol, n_lo, Tpad,
    srcidxs, dlocs, dlocTs): srcidx is the per-core [128, Tpad*8] int16 SBUF
    image; dloc is [P, Tpad] f32 dst-slot per edge slot (buffer order, -1
    pad); dlocT is [1, Tpad*128] bf16 dst-slot along free dim (buffer order).
    """
    NHALF = npad // 2
    percore = []
    cnt_bh = np.zeros((nb, 2), np.int64)
    for c in range(NCORES):
        m = (dst_all // per) == c
        s = src_all[m]
        loc = dst_all[m] - c * per
        b = loc // P
        sc = s // per
        srem = s % per
        hf = (srem // P >= _ba(nb)).astype(np.int64)
        order = np.lexsort((hf, b))
        s, loc, b, hf = s[order], loc[order], b[order], hf[order]
        sc, srem = sc[order], srem[order]
        cnt = np.bincount(b * 2 + hf, minlength=nb * 2).reshape(nb, 2)
        cnt_bh = np.maximum(cnt_bh, cnt)
        ba = _ba(nb)
        rloc = np.where(srem < ba * P, sc * (ba * P) + srem,
                        sc * (per - ba * P) + srem - ba * P)
        percore.append((rloc, loc, b, hf, cnt))

    tiles_bh = (cnt_bh + P - 1) // P            # [nb, 2]
    # processing-order tiles
    blk_l, hf_l = [], []
    tstart = np.zeros((nb, 2), np.int64)
    t = 0
    for b in range(nb):
        for hf in (0, 1):
            tstart[b, hf] = t
            n = int(tiles_bh[b, hf])
            blk_l += [b] * n
            hf_l += [hf] * n
            t += n
    T = t
    nchunks = (T + CT - 1) // CT
    Tpad = nchunks * CT
    blk_of_tile = np.array(blk_l + [nb - 1] * (Tpad - T), np.int64)
    hf_of_tile = np.array(hf_l + [0] * (Tpad - T), np.int64)
    start_t = np.zeros(Tpad, bool)
    stop_t = np.zeros(Tpad, bool)
    for b in range(nb):
        w = np.nonzero(blk_of_tile == b)[0]
        start_t[w[0]] = True
        stop_t[w[-1]] = True

    # buffer-column mapping: per chunk, lo tiles first then hi tiles
    bufcol = np.zeros(Tpad, np.int64)
    n_lo = []
    for ch in range(nchunks):
        ts = np.arange(ch * CT, (ch + 1) * CT)
        lo = ts[hf_of_tile[ts] == 0]
        hi = ts[hf_of_tile[ts] == 1]
        bufcol[lo] = np.arange(len(lo))
        bufcol[hi] = len(lo) + np.arange(len(hi))
        n_lo.append(len(lo))
    g2b = (np.arange(Tpad) // CT) * CT + bufcol   # proc tile -> buffer col

    import ml_dtypes
    SW = Tpad * 8
    srcidxs, dlocs, dlocTs = [], [], []
    for c in range(NCORES):
        rloc, loc, b, hf, cnt = percore[c]
        srcv = np.zeros((P, Tpad), np.int64)
        dlp = np.full((P, Tpad), -1.0, np.float32)   # proc order
        off = np.zeros(nb * 2 + 1, np.int64)
        off[1:] = np.cumsum(cnt.reshape(-1))
        key = b * 2 + hf
        pos = np.arange(len(rloc)) - off[key]
        tt = tstart[b, hf] + pos // P
        pp = pos % P
        srcv[pp, tt] = rloc
        dlp[pp, tt] = loc % P
        # sbuf idx image: slot (p, t) -> row 16k+p%16, col (bufgcol*8)+p//16
        p_g, t_g = np.mgrid[0:P, 0:Tpad]
        col = g2b[t_g] * 8 + p_g // 16
        row = p_g % 16
        si = np.zeros((P, SW), np.int16)
        for k in range(8):
            si[16 * k + row, col] = srcv
        srcidxs.append(si)
        dloc_buf = np.full((P, Tpad), -1.0, np.float32)
        dloc_buf[:, g2b] = dlp
        dlocs.append(dloc_buf.astype(ml_dtypes.bfloat16))
        dT = np.full((Tpad, P), -1.0, np.float32)
        dT[g2b, :] = dlp.T
        dlocTs.append(dT.reshape(1, Tpad * P).astype(ml_dtypes.bfloat16))
    return (blk_of_tile, start_t, stop_t, bufcol, n_lo, Tpad,
            srcidxs, dlocs, dlocTs)


def _build(npad, Tpad, blk_of_tile, start_t, stop_t, bufcol, n_lo):
    per = npad // NCORES
    nb = per // P
    NHALF = npad // 2
    nlayers = 3
    nchunks = Tpad // CT
    f32 = mybir.dt.float32
    bf16 = mybir.dt.bfloat16
    i16 = mybir.dt.int16

    nc = bass.Bass(num_devices=NCORES, num_swdge_queues=NSWQ)
    # ---- dram I/O
    xT_d = nc.dram_tensor("xT", [P, per], bf16, kind="ExternalInput")
    waug_d = nc.dram_tensor("waug", [nlayers, P, AUGW], bf16, kind="ExternalInput")
    wh_d = nc.dram_tensor("wh", [P, OUT], f32, kind="ExternalInput")
    iota_d = nc.dram_tensor("iota", [P, 4 * P], f32, kind="ExternalInput")
    iotap_d = nc.dram_tensor("iotap", [P, 1], bf16, kind="ExternalInput")
    ident_d = nc.dram_tensor("ident", [P, P], f32, kind="ExternalInput")
    SW = Tpad * 8
    srcidx_d = nc.dram_tensor("srcidx", [P, SW], i16, kind="ExternalInput")
    dloc_d = nc.dram_tensor("dloc", [P, Tpad], bf16, kind="ExternalInput")
    iotarep_d = nc.dram_tensor("iotarep", [P, CT * P], bf16, kind="ExternalInput")
    dlocT_d = nc.dram_tensor("dlocT", [1, Tpad * P], bf16, kind="ExternalInput")
    batchf_d = nc.dram_tensor("batchf", [P, nb], f32, kind="ExternalInput")
    out_d = nc.dram_tensor("out", [GRAPHS, OUT], f32, kind="ExternalOutput")

    h_loc = [nc.dram_tensor(f"h_loc{l}", [per, TW], bf16) for l in range(nlayers)]
    h_full = [nc.dram_tensor(f"h_full{l}", [npad, TW], bf16, addr_space="Shared")
              for l in range(nlayers)]

    groups = [list(range(NCORES))]

    with ExitStack() as ctx:
        tc = ctx.enter_context(tile.TileContext(nc))
        sb = ctx.enter_context(tc.tile_pool(name="sb", bufs=1))
        sb_g = ctx.enter_context(tc.tile_pool(name="sbg", bufs=3))
        sb_s = ctx.enter_context(tc.tile_pool(name="sbs", bufs=3))
        sb_t = ctx.enter_context(tc.tile_pool(name="sbt", bufs=3))
        sb_w = ctx.enter_context(tc.tile_pool(name="sbw", bufs=3))
        ps_h = ctx.enter_context(tc.tile_pool(name="psh", bufs=2, space="PSUM"))
        ps_agg = ctx.enter_context(tc.tile_pool(name="psagg", bufs=2, space="PSUM"))
        ps_ad = ctx.enter_context(tc.tile_pool(name="psad", bufs=2, space="PSUM"))
        ps_xp = ctx.enter_context(tc.tile_pool(name="psxp", bufs=1, space="PSUM"))
        ps_fin = ctx.enter_context(tc.tile_pool(name="psfin", bufs=1, space="PSUM"))

        # ---- persistent SBUF state
        xT = sb.tile([P, per], bf16)
        nc.sync.dma_start(out=xT[:], in_=xT_d[:])
        waug = sb.tile([P, nlayers, AUGW], bf16)
        nc.sync.dma_start(out=waug[:],
                          in_=waug_d[:].rearrange("l p a -> p l a"))
        wh = sb.tile([P, OUT], f32)
        nc.sync.dma_start(out=wh[:], in_=wh_d[:])
        iota = sb.tile([P, 4, P], f32)
        nc.sync.dma_start(out=iota[:].rearrange("p a b -> p (a b)"), in_=iota_d[:])
        iotap = sb.tile([P, 1], bf16)
        nc.sync.dma_start(out=iotap[:], in_=iotap_d[:])
        srci = sb.tile([P, SW], i16)
        nc.sync.dma_start(out=srci[:], in_=srcidx_d[:])
        dloc = sb.tile([P, Tpad, 1], bf16)
        nc.sync.dma_start(out=dloc[:].rearrange("p t o -> p (t o)"), in_=dloc_d[:])
        iotarep = sb.tile([P, CT, P], bf16)
        nc.sync.dma_start(out=iotarep[:].rearrange("p c e -> p (c e)"),
                          in_=iotarep_d[:])
        batchf = sb.tile([P, nb, 1], f32)
        nc.sync.dma_start(out=batchf[:].rearrange("p b o -> p (b o)"), in_=batchf_d[:])
        identf = sb.tile([P, P], f32)
        nc.sync.dma_start(out=identf[:], in_=ident_d[:])

        nc.gpsimd.load_library(library_config.mlp)

        _regs = {}

        def nreg(v):
            if v not in _regs:
                _regs[v] = nc.gpsimd.to_reg(v)
            return _regs[v]

        hsb2 = [sb.tile([P, nb, TBL], bf16, name=f"hsb{i}") for i in range(2)]
        adsb2 = [sb.tile([P, nb, H], bf16, name=f"adsb{i}") for i in range(2)]
        pooled_ps = ps_fin.tile([GRAPHS, HC], f32)
        qn = [0]

        def nextq():
            qn[0] = (qn[0] + 1) % NSWQ
            return qn[0]

        ba = _ba(nb)
        RA = ba * P            # piece-A local rows
        GA = NCORES * RA       # piece-A global rows
        dhA = [None] * 3
        dhB = [None] * 3
        ccA = [None] * 3
        ccB = [None] * 3

        def node_mm(l, b):
            hsb, adsb = hsb2[l % 2], adsb2[l % 2]
            ps = ps_h.tile([P, AUGW], f32, tag="ndps")
            nc.tensor.matmul(ps[:], lhsT=xT[:, b * P:(b + 1) * P],
                             rhs=waug[:, l, :], start=True, stop=True)
            nc.vector.tensor_copy(out=hsb[:, b, :], in_=ps[:, :TBL])
            nc.vector.tensor_copy(out=adsb[:, b, :], in_=ps[:, TBL:AUGW])
            if b == ba - 1:
                dhA[l] = nc.sync.dma_start(
                    out=h_loc[l][0:RA, 0:TBL].rearrange("(b p) d -> p b d", p=P),
                    in_=hsb[:, 0:ba, :])
            if b == nb - 1:
                dhB[l] = nc.sync.dma_start(
                    out=h_loc[l][RA:per, 0:TBL].rearrange("(b p) d -> p b d", p=P),
                    in_=hsb[:, ba:nb, :])

        def emit_ccA(l):
            ccA[l] = nc.gpsimd.collective_compute(
                "AllGather", mybir.AluOpType.bypass, replica_groups=groups,
                ins=[h_loc[l][0:RA, :]], outs=[h_full[l][0:GA, :]])
            add_dep_helper(ccA[l].ins, dhA[l].ins, sync=True, reason="hA before ag")

        def emit_ccB(l):
            ccB[l] = nc.gpsimd.collective_compute(
                "AllGather", mybir.AluOpType.bypass, replica_groups=groups,
                ins=[h_loc[l][RA:per, :]], outs=[h_full[l][GA:npad, :]])
            add_dep_helper(ccB[l].ins, dhB[l].ins, sync=True, reason="hB before ag")

        # layer-0 node phase prologue
        for b in range(nb):
            node_mm(0, b)
        emit_ccA(0)
        emit_ccB(0)

        for l in range(3):
            adsb = adsb2[l % 2]
            # ===== edge phase =====
            agg_of_blk = {}
            for ci in range(nchunks):
                t0 = ci * CT
                nlo = int(n_lo[ci])
                if (l < 2 and ccA[l + 1] is None and dhA[l + 1] is not None
                        and ci >= CC_HIDE_CHUNK):
                    emit_ccA(l + 1)
                g = sb_g.tile([P, CT, TW], bf16, tag="gath")
                if nlo > 0:
                    glo = nc.gpsimd.dma_gather(
                        out_ap=g[:, 0:nlo, :], in_ap=h_full[l][0:GA, :],
                        idxs_ap=srci[:, t0 * 8:t0 * 8 + nlo * 8],
                        num_idxs=nlo * P, num_idxs_reg=nreg(nlo * P), elem_size=TW,
                        single_packet=False, queue_num=nextq())
                    add_dep_helper(glo.ins, ccA[l].ins, sync=True, reason="gather after agA")
                if nlo < CT:
                    ghi = nc.gpsimd.dma_gather(
                        out_ap=g[:, nlo:CT, :], in_ap=h_full[l][GA:npad, :],
                        idxs_ap=srci[:, t0 * 8 + nlo * 8:(t0 + CT) * 8],
                        num_idxs=(CT - nlo) * P, num_idxs_reg=nreg((CT - nlo) * P),
                        elem_size=TW, single_packet=False, queue_num=nextq())
                    add_dep_helper(ghi.ins, ccB[l].ins, sync=True, reason="gather after agB")

                # transposed selectors (node-slot one-hot along partitions)
                dT = sb_t.tile([P, CT, P], bf16, tag="dT")
                nc.sync.dma_start(
                    out=dT[:].rearrange("p c e -> p (c e)"),
                    in_=dlocT_d[0:1, t0 * P:(t0 + CT) * P].to_broadcast(
                        [P, CT * P]))
                selT = sb_t.tile([P, CT, P], bf16, tag="selT")
                nc.vector.tensor_tensor(
                    out=selT[:], in0=dT[:],
                    in1=iotap[:].rearrange("p (c e) -> p c e", c=1).to_broadcast(
                        [P, CT, P]),
                    op=mybir.AluOpType.is_equal)
                # per-edge alpha_dst via PE: adps[:, j, :] = selT_j^T @ adsb[b]
                adps = ps_ad.tile([P, CT, H], f32, tag="adps")
                for t in range(t0, t0 + CT):
                    b = int(blk_of_tile[t])
                    j = int(bufcol[t])
                    nc.tensor.matmul(adps[:, j, :], lhsT=selT[:, j, :],
                                     rhs=adsb[:, b, :], start=True, stop=True)

                # e = lrelu(a_s + a_d); w = exp(e) written into g cols HC:TBL
                lg = sb_w.tile([P, CT, H], f32, tag="lg")
                nc.vector.tensor_tensor(out=lg[:], in0=adps[:],
                                        in1=g[:, :, HC:TBL],
                                        op=mybir.AluOpType.add)
                lr = sb_w.tile([P, CT, H], f32, tag="lr")
                if K_LRELU:
                    nc.scalar.activation(lr[:], lg[:],
                                         mybir.ActivationFunctionType.Lrelu,
                                         alpha=NEG_SLOPE)
                else:
                    nc.vector.tensor_scalar_mul(lr[:], lg[:], NEG_SLOPE)
                    nc.vector.tensor_tensor(out=lr[:], in0=lr[:], in1=lg[:],
                                            op=mybir.AluOpType.max)
                nc.scalar.activation(g[:, :, HC:TBL], lr[:],
                                     mybir.ActivationFunctionType.Exp)
                # msg in place: g[:, :, h*Ch:(h+1)*Ch] *= w[h]
                nc.vector.tensor_tensor(
                    out=g[:, :, 0:HC].rearrange("p c (h w) -> p c h w", h=H),
                    in0=g[:, :, 0:HC].rearrange("p c (h w) -> p c h w", h=H),
                    in1=g[:, :, HC:TBL].rearrange("p c (h o) -> p c h o", o=1)
                        .to_broadcast([P, CT, H, Ch]),
                    op=mybir.AluOpType.mult)
                # selectors for the whole chunk (dst one-hot per edge)
                sel = sb_s.tile([P, CT, P], bf16, tag="sel")
                nc.vector.tensor_tensor(
                    out=sel[:], in0=iotarep[:],
                    in1=dloc[:, t0:t0 + CT, :].to_broadcast([P, CT, P]),
                    op=mybir.AluOpType.is_equal)
                # aggregate per tile (processing order)
                for t in range(t0, t0 + CT):
                    b = int(blk_of_tile[t])
                    j = int(bufcol[t])
                    if start_t[t]:
                        agg_of_blk[b] = ps_agg.tile([P, TBL], f32, tag="agg",
                                                    name=f"agg{l}_{b}")
                    nc.tensor.matmul(agg_of_blk[b][:], lhsT=sel[:, j, :],
                                     rhs=g[:, j, 0:TBL],
                                     start=bool(start_t[t]),
                                     stop=bool(stop_t[t]))
                    if stop_t[t]:
                        agg = agg_of_blk.pop(b)
                        rec = sb_w.tile([P, H], f32, tag="rec")
                        nc.vector.reciprocal(rec[:], agg[:, HC:TBL])
                        xb = sb_w.tile([P, HC], f32, tag="xb")
                        nc.vector.tensor_tensor(
                            out=xb[:].rearrange("p (h w) -> p h w", h=H),
                            in0=agg[:, 0:HC].rearrange("p (h w) -> p h w", h=H),
                            in1=rec[:].rearrange("p (h o) -> p h o", o=1)
                                .to_broadcast([P, H, Ch]),
                            op=mybir.AluOpType.mult)
                        if K_RELU:
                            nc.scalar.activation(xb[:], xb[:],
                                                 mybir.ActivationFunctionType.Relu)
                        else:
                            nc.vector.tensor_scalar_max(xb[:], xb[:], 0.0)
                        if l < 2:
                            xps = ps_xp.tile([P, P], f32, tag="xps")
                            nc.tensor.transpose(xps[:], xb[:], identf[:])
                            nc.vector.tensor_copy(
                                out=xT[:, b * P:(b + 1) * P], in_=xps[:])
                            if b >= 2:
                                node_mm(l + 1, b - 2)
                            if b == nb - 1:
                                node_mm(l + 1, nb - 2)
                                node_mm(l + 1, nb - 1)
                        else:
                            bsel = sb_w.tile([P, GRAPHS], f32, tag="bsel")
                            nc.vector.tensor_tensor(
                                out=bsel[:],
                                in0=batchf[:, b, :].to_broadcast([P, GRAPHS]),
                                in1=iota[:, 0, :GRAPHS],
                                op=mybir.AluOpType.is_equal)
                            nc.tensor.matmul(pooled_ps[:], lhsT=bsel[:],
                                             rhs=xb[:], start=(b == 0),
                                             stop=(b == nb - 1))
            if l < 2:
                if ccA[l + 1] is None:
                    emit_ccA(l + 1)
                emit_ccB(l + 1)

        # ===== head =====
        pooled_sb = sb.tile([GRAPHS, HC], f32)
        nc.vector.tensor_copy(out=pooled_sb[:], in_=pooled_ps[:])
        pT_ps = ps_xp.tile([P, GRAPHS], f32, tag="xps")
        nc.tensor.transpose(pT_ps[:], pooled_sb[:], identf[:GRAPHS, :GRAPHS])
        pT_sb = sb.tile([P, GRAPHS], f32)
        nc.vector.tensor_copy(out=pT_sb[:], in_=pT_ps[:])
        log_ps = ps_xp.tile([GRAPHS, OUT], f32, tag="xps")
        nc.tensor.matmul(log_ps[:], lhsT=pT_sb[:], rhs=wh[:], start=True, stop=True)
        log_sb = sb.tile([GRAPHS, OUT], f32)
        nc.vector.tensor_copy(out=log_sb[:], in_=log_ps[:])
        nc.sync.dma_start(out=out_d[:], in_=log_sb[:])

    _fixup_wait_limits(nc)
    mybir.codegen_inst_isa_subclasses(nc)
    return nc


def prepare(x, Ws, a_srcs, a_dsts, biases, Wh, bh, edge_index, batch):
    n = x.shape[0]
    npad = int(math.ceil(n / (NCORES * P)) * NCORES * P)
    per = npad // NCORES
    nb = per // P

    x = np.asarray(x, np.float32)
    Ws = [np.asarray(w, np.float32) for w in Ws]
    a_srcs = [np.asarray(a, np.float32) for a in a_srcs]
    a_dsts = [np.asarray(a, np.float32) for a in a_dsts]
    Wh = np.asarray(Wh, np.float32)
    bh = np.asarray(bh, np.float32)
    edge_index = np.asarray(edge_index)
    batch = np.asarray(batch)
    for b in biases:
        assert np.allclose(np.asarray(b), 0.0), "nonzero GAT biases unsupported"

    import ml_dtypes
    # W_aug = [W | W@As | W@Ad]
    waugs = []
    for l in range(3):
        As = np.zeros((HC, H), np.float32)
        Ad = np.zeros((HC, H), np.float32)
        for h in range(H):
            As[h * Ch:(h + 1) * Ch, h] = a_srcs[l][h]
            Ad[h * Ch:(h + 1) * Ch, h] = a_dsts[l][h]
        W = Ws[l]
        waugs.append(np.concatenate([W, W @ As, W @ Ad], axis=1))
    waug = np.stack(waugs, 0).astype(ml_dtypes.bfloat16)  # [3, 128, AUGW]

    # edges + self loops (incl. pad nodes, so every row has >=1 edge)
    src_all = np.concatenate([edge_index[0].astype(np.int64),
                              np.arange(npad, dtype=np.int64)])
    dst_all = np.concatenate([edge_index[1].astype(np.int64),
                              np.arange(npad, dtype=np.int64)])
    (blk_of_tile, start_t, stop_t, bufcol, n_lo, Tpad,
     srcidxs, dlocs, dlocTs) = _prep_edges(src_all, dst_all, per, nb, npad)

    xpad = np.zeros((npad, HC), np.float32)
    xpad[:n] = x
    iota = np.tile(np.arange(P, dtype=np.float32)[None, :], (P, 4))

    batchf_full = np.full(npad, -1.0, np.float32)
    batchf_full[:n] = batch.astype(np.float32)

    nc = _build(npad, Tpad, blk_of_tile, start_t, stop_t, bufcol, n_lo)

    in_maps = []
    for c in range(NCORES):
        sl = slice(c * per, (c + 1) * per)
        in_maps.append({
            "xT": np.ascontiguousarray(xpad[sl].T).astype(ml_dtypes.bfloat16),
            "waug": waug,
            "wh": Wh,
            "iota": iota,
            "iotap": np.arange(P, dtype=np.float32)[:, None].astype(
                ml_dtypes.bfloat16),
            "ident": np.eye(P, dtype=np.float32),
            "srcidx": srcidxs[c],
            "dloc": dlocs[c],
            "iotarep": np.tile(np.arange(P, dtype=np.float32)[None, :],
                               (P, CT)).astype(ml_dtypes.bfloat16),
            "dlocT": dlocTs[c],
            "batchf": np.ascontiguousarray(
                batchf_full[sl].reshape(nb, P).T),
            })
    return nc, in_maps


def run_gat(x, Ws, a_srcs, a_dsts, biases, Wh, bh, edge_index, batch):
    nc, in_maps = prepare(x, Ws, a_srcs, a_dsts, biases, Wh, bh,
                          edge_index, batch)
    res = run_bass_kernel_spmd(nc, in_maps, list(range(NCORES)))
    global LAST_EXEC_NS
    LAST_EXEC_NS = getattr(res, "exec_time_ns", None)
    logits = np.zeros((GRAPHS, OUT), np.float32)
    for c in range(NCORES):
        logits += res.results[c]["out"]
    return logits + bh


def kernel(**inputs):
    return np.asarray(run_gat(
        inputs["x"], inputs["Ws"], inputs["a_srcs"], inputs["a_dsts"],
        inputs["biases"], inputs["Wh"], inputs["bh"], inputs["edge_index"],
        inputs["batch"]), np.float32)


# revision 28
# speedup vs baseline: 1.1088x; 1.0496x over previous
"""Trainium2 Bass kernel for 3-layer GAT + global_add_pool + linear head.

Design (v3):
- Nodes (and incoming edges) sharded across 8 cores by dst.
- Node phase per layer: h_aug = x @ [W | W@As | W@Ad] on PE (bf16 in, f32 out).
  [h|alpha_src] rows (bf16, 512B stride) go to a DRAM table that is
  AllGathered across cores.
- Edge phase: edges tiled 128/dst-block (sub-split by src half for int16
  gather indices); per chunk of CT tiles, batched InstDMAGatherAnt fetches
  h[src] rows on 4 SWDGE queues. Per-edge alpha_dst comes from transposed
  selector matmuls on the (otherwise idle) PE; selectors are built by vector
  is_equal against a streamed dst-slot pattern.
- e = lrelu(a_s+a_d) on vector (max(z,.2z)), exp batched on scalar, messages
  scaled in place (bf16), then per-tile selector matmuls accumulate [w*h | w]
  into PSUM per dst block; normalization + relu on block end.
- Partial pooled logits summed on host.

Self-contained: no file reads; shapes hardcoded via constants.
"""
import math
import numpy as np
from contextlib import ExitStack

import concourse.bass as bass
import concourse.mybir as mybir
import concourse.tile as tile
from concourse.bass_utils import run_bass_kernel_spmd
from concourse.tile_rust import add_dep_helper
from concourse import library_config

NCORES = 8
P = 128
H = 4
Ch = 32
HC = 128
AUGW = HC + 2 * H   # 136: node matmul out [h | a_src | a_dst]
TBL = HC + H        # 132: useful row [h | a_src]
TW = 256            # bf16 table row width (512B stride)
NEG_SLOPE = 0.2
GRAPHS = 64
OUT = 10
CT = 32             # tiles (of 128 edges) per chunk
NSWQ = 4            # SWDGE queues
BA = 32             # blocks in table piece A (per core), capped to nb//2


def _ba(nb):
    return max(1, min(BA, nb // 2))
CC_HIDE_CHUNK = 20  # edge-phase chunk index at which next layer's AG-A issues
import os as _os
K_LRELU = _os.environ.get("K_LRELU", "0") == "1"
K_RELU = _os.environ.get("K_RELU", "1") == "1"

# instruction types whose BIR struct cannot carry all Tile-emitted waits
_WAIT_CAPS = {
    "InstDMAGatherAnt": 0,
    "InstDMAScatterAddAnt": 0,
    "InstNoOp": 1,
    "InstDrain": 1,
    "InstCollectiveCompute": 1,
}


def _fixup_wait_limits(nc):
    k = 0
    for fn in nc.m.functions:
        for blk in fn.blocks:
            out = []
            for inst in blk.instructions:
                cap = _WAIT_CAPS.get(type(inst).__name__, 1)
                si = inst.sync_info
                if si is not None:
                    waits = list(si.on_wait)
                    if len(waits) > cap:
                        keep, move = waits[:cap], waits[cap:]
                        for w in move:
                            nop = mybir.InstNoOp(name=f"waitfix_{k}", text_hint="wait_fixup")
                            k += 1
                            nop.engine = inst.engine
                            nop.sync_info = type(si)(on_wait=[w], on_update=[])
                            out.append(nop)
                        inst.sync_info = type(si)(on_wait=list(keep), on_update=list(si.on_update))
                out.append(inst)
            blk.instructions = out
    return k


def _prep_edges(src_all, dst_all, per, nb, npad):
    """Tile edges per core by (dst block, src half), build gather idx arrays.

    Returns (blk_of_tile, start_t, stop_t, bufcol, n_lo, Tpad,
    srcidxs, dlocs, dlocTs): srcidx is the per-core [128, Tpad*8] int16 SBUF
    image; dloc is [P, Tpad] f32 dst-slot per edge slot (buffer order, -1
    pad); dlocT is [1, Tpad*128] bf16 dst-slot along free dim (buffer order).
    """
    NHALF = npad // 2
    percore = []
    cnt_bh = np.zeros((nb, 2), np.int64)
    for c in range(NCORES):
        m = (dst_all // per) == c
        s = src_all[m]
        loc = dst_all[m] - c * per
        b = loc // P
        sc = s // per
        srem = s % per
        hf = (srem // P >= _ba(nb)).astype(np.int64)
        order = np.lexsort((hf, b))
        s, loc, b, hf = s[order], loc[order], b[order], hf[order]
        sc, srem = sc[order], srem[order]
        cnt = np.bincount(b * 2 + hf, minlength=nb * 2).reshape(nb, 2)
        cnt_bh = np.maximum(cnt_bh, cnt)
        ba = _ba(nb)
        rloc = np.where(srem < ba * P, sc * (ba * P) + srem,
                        sc * (per - ba * P) + srem - ba * P)
        percore.append((rloc, loc, b, hf, cnt))

    tiles_bh = (cnt_bh + P - 1) // P            # [nb, 2]
    # processing-order tiles
    blk_l, hf_l = [], []
    tstart = np.zeros((nb, 2), np.int64)
    t = 0
    for b in range(nb):
        for hf in (0, 1):
            tstart[b, hf] = t
            n = int(tiles_bh[b, hf])
            blk_l += [b] * n
            hf_l += [hf] * n
            t += n
    T = t
    nchunks = (T + CT - 1) // CT
    Tpad = nchunks * CT
    blk_of_tile = np.array(blk_l + [nb - 1] * (Tpad - T), np.int64)
    hf_of_tile = np.array(hf_l + [0] * (Tpad - T), np.int64)
    start_t = np.zeros(Tpad, bool)
    stop_t = np.zeros(Tpad, bool)
    for b in range(nb):
        w = np.nonzero(blk_of_tile == b)[0]
        start_t[w[0]] = True
        stop_t[w[-1]] = True

    # buffer-column mapping: per chunk, lo tiles first then hi tiles
    bufcol = np.zeros(Tpad, np.int64)
    n_lo = []
    for ch in range(nchunks):
        ts = np.arange(ch * CT, (ch + 1) * CT)
        lo = ts[hf_of_tile[ts] == 0]
        hi = ts[hf_of_tile[ts] == 1]
        bufcol[lo] = np.arange(len(lo))
        bufcol[hi] = len(lo) + np.arange(len(hi))
        n_lo.append(len(lo))
    g2b = (np.arange(Tpad) // CT) * CT + bufcol   # proc tile -> buffer col

    import ml_dtypes
    SW = Tpad * 8
    srcidxs, dlocs, dlocTs = [], [], []
    for c in range(NCORES):
        rloc, loc, b, hf, cnt = percore[c]
        srcv = np.zeros((P, Tpad), np.int64)
        dlp = np.full((P, Tpad), -1.0, np.float32)   # proc order
        off = np.zeros(nb * 2 + 1, np.int64)
        off[1:] = np.cumsum(cnt.reshape(-1))
        key = b * 2 + hf
        pos = np.arange(len(rloc)) - off[key]
        tt = tstart[b, hf] + pos // P
        pp = pos % P
        srcv[pp, tt] = rloc
        dlp[pp, tt] = loc % P
        # sbuf idx image: slot (p, t) -> row 16k+p%16, col (bufgcol*8)+p//16
        p_g, t_g = np.mgrid[0:P, 0:Tpad]
        col = g2b[t_g] * 8 + p_g // 16
        row = p_g % 16
        si = np.zeros((P, SW), np.int16)
        for k in range(8):
            si[16 * k + row, col] = srcv
        srcidxs.append(si)
        dloc_buf = np.full((P, Tpad), -1.0, np.float32)
        dloc_buf[:, g2b] = dlp
        dlocs.append(dloc_buf.astype(ml_dtypes.bfloat16))
        dT = np.full((Tpad, P), -1.0, np.float32)
        dT[g2b, :] = dlp.T
        dlocTs.append(dT.reshape(1, Tpad * P).astype(np.int8))
    return (blk_of_tile, start_t, stop_t, bufcol, n_lo, Tpad,
            srcidxs, dlocs, dlocTs)


def _build(npad, Tpad, blk_of_tile, start_t, stop_t, bufcol, n_lo):
    per = npad // NCORES
    nb = per // P
    NHALF = npad // 2
    nlayers = 3
    nchunks = Tpad // CT
    f32 = mybir.dt.float32
    bf16 = mybir.dt.bfloat16
    i16 = mybir.dt.int16

    nc = bass.Bass(num_devices=NCORES, num_swdge_queues=NSWQ)
    # ---- dram I/O
    xT_d = nc.dram_tensor("xT", [P, per], bf16, kind="ExternalInput")
    waug_d = nc.dram_tensor("waug", [nlayers, P, AUGW], bf16, kind="ExternalInput")
    wh_d = nc.dram_tensor("wh", [P, OUT], f32, kind="ExternalInput")
    iota_d = nc.dram_tensor("iota", [P, 4 * P], f32, kind="ExternalInput")
    iotap_d = nc.dram_tensor("iotap", [P, 1], mybir.dt.int8, kind="ExternalInput")
    ident_d = nc.dram_tensor("ident", [P, P], f32, kind="ExternalInput")
    SW = Tpad * 8
    srcidx_d = nc.dram_tensor("srcidx", [P, SW], i16, kind="ExternalInput")
    dloc_d = nc.dram_tensor("dloc", [P, Tpad], bf16, kind="ExternalInput")
    iotarep_d = nc.dram_tensor("iotarep", [P, CT * P], bf16, kind="ExternalInput")
    dlocT_d = nc.dram_tensor("dlocT", [1, Tpad * P], mybir.dt.int8, kind="ExternalInput")
    batchf_d = nc.dram_tensor("batchf", [P, nb], f32, kind="ExternalInput")
    out_d = nc.dram_tensor("out", [GRAPHS, OUT], f32, kind="ExternalOutput")

    h_loc = [nc.dram_tensor(f"h_loc{l}", [per, TW], bf16) for l in range(nlayers)]
    h_full = [nc.dram_tensor(f"h_full{l}", [npad, TW], bf16, addr_space="Shared")
              for l in range(nlayers)]

    groups = [list(range(NCORES))]

    with ExitStack() as ctx:
        tc = ctx.enter_context(tile.TileContext(nc))
        sb = ctx.enter_context(tc.tile_pool(name="sb", bufs=1))
        sb_g = ctx.enter_context(tc.tile_pool(name="sbg", bufs=3))
        sb_s = ctx.enter_context(tc.tile_pool(name="sbs", bufs=3))
        sb_t = ctx.enter_context(tc.tile_pool(name="sbt", bufs=3))
        sb_w = ctx.enter_context(tc.tile_pool(name="sbw", bufs=3))
        ps_h = ctx.enter_context(tc.tile_pool(name="psh", bufs=2, space="PSUM"))
        ps_agg = ctx.enter_context(tc.tile_pool(name="psagg", bufs=2, space="PSUM"))
        ps_ad = ctx.enter_context(tc.tile_pool(name="psad", bufs=2, space="PSUM"))
        ps_xp = ctx.enter_context(tc.tile_pool(name="psxp", bufs=1, space="PSUM"))
        ps_fin = ctx.enter_context(tc.tile_pool(name="psfin", bufs=1, space="PSUM"))

        # ---- persistent SBUF state
        xT = sb.tile([P, per], bf16)
        nc.sync.dma_start(out=xT[:], in_=xT_d[:])
        waug = sb.tile([P, nlayers, AUGW], bf16)
        nc.sync.dma_start(out=waug[:],
                          in_=waug_d[:].rearrange("l p a -> p l a"))
        wh = sb.tile([P, OUT], f32)
        nc.sync.dma_start(out=wh[:], in_=wh_d[:])
        iota = sb.tile([P, 4, P], f32)
        nc.sync.dma_start(out=iota[:].rearrange("p a b -> p (a b)"), in_=iota_d[:])
        iotap = sb.tile([P, 1], mybir.dt.int8)
        nc.sync.dma_start(out=iotap[:], in_=iotap_d[:])
        srci = sb.tile([P, SW], i16)
        nc.sync.dma_start(out=srci[:], in_=srcidx_d[:])
        dloc = sb.tile([P, Tpad, 1], bf16)
        nc.sync.dma_start(out=dloc[:].rearrange("p t o -> p (t o)"), in_=dloc_d[:])
        iotarep = sb.tile([P, CT, P], bf16)
        nc.sync.dma_start(out=iotarep[:].rearrange("p c e -> p (c e)"),
                          in_=iotarep_d[:])
        batchf = sb.tile([P, nb, 1], f32)
        nc.sync.dma_start(out=batchf[:].rearrange("p b o -> p (b o)"), in_=batchf_d[:])
        identf = sb.tile([P, P], f32)
        nc.sync.dma_start(out=identf[:], in_=ident_d[:])

        nc.gpsimd.load_library(library_config.mlp)

        _regs = {}

        def nreg(v):
            if v not in _regs:
                _regs[v] = nc.gpsimd.to_reg(v)
            return _regs[v]

        hsb2 = [sb.tile([P, nb, TBL], bf16, name=f"hsb{i}") for i in range(2)]
        adsb2 = [sb.tile([P, nb, H], bf16, name=f"adsb{i}") for i in range(2)]
        pooled_ps = ps_fin.tile([GRAPHS, HC], f32)
        qn = [0]

        def nextq():
            qn[0] = (qn[0] + 1) % NSWQ
            return qn[0]

        ba = _ba(nb)
        RA = ba * P            # piece-A local rows
        GA = NCORES * RA       # piece-A global rows
        dhA = [None] * 3
        dhB = [None] * 3
        ccA = [None] * 3
        ccB = [None] * 3

        def node_mm(l, b):
            hsb, adsb = hsb2[l % 2], adsb2[l % 2]
            ps = ps_h.tile([P, AUGW], f32, tag="ndps")
            nc.tensor.matmul(ps[:], lhsT=xT[:, b * P:(b + 1) * P],
                             rhs=waug[:, l, :], start=True, stop=True)
            nc.vector.tensor_copy(out=hsb[:, b, :], in_=ps[:, :TBL])
            nc.vector.tensor_copy(out=adsb[:, b, :], in_=ps[:, TBL:AUGW])
            if b == ba - 1:
                dhA[l] = nc.sync.dma_start(
                    out=h_loc[l][0:RA, 0:TBL].rearrange("(b p) d -> p b d", p=P),
                    in_=hsb[:, 0:ba, :])
            if b == nb - 1:
                dhB[l] = nc.sync.dma_start(
                    out=h_loc[l][RA:per, 0:TBL].rearrange("(b p) d -> p b d", p=P),
                    in_=hsb[:, ba:nb, :])

        def emit_ccA(l):
            ccA[l] = nc.gpsimd.collective_compute(
                "AllGather", mybir.AluOpType.bypass, replica_groups=groups,
                ins=[h_loc[l][0:RA, :]], outs=[h_full[l][0:GA, :]])
            add_dep_helper(ccA[l].ins, dhA[l].ins, sync=True, reason="hA before ag")

        def emit_ccB(l):
            ccB[l] = nc.gpsimd.collective_compute(
                "AllGather", mybir.AluOpType.bypass, replica_groups=groups,
                ins=[h_loc[l][RA:per, :]], outs=[h_full[l][GA:npad, :]])
            add_dep_helper(ccB[l].ins, dhB[l].ins, sync=True, reason="hB before ag")

        # layer-0 node phase prologue
        for b in range(nb):
            node_mm(0, b)
        emit_ccA(0)
        emit_ccB(0)

        for l in range(3):
            adsb = adsb2[l % 2]
            # ===== edge phase =====
            agg_of_blk = {}
            for ci in range(nchunks):
                t0 = ci * CT
                nlo = int(n_lo[ci])
                if (l < 2 and ccA[l + 1] is None and dhA[l + 1] is not None
                        and ci >= CC_HIDE_CHUNK):
                    emit_ccA(l + 1)
                g = sb_g.tile([P, CT, TW], bf16, tag="gath")
                if nlo > 0:
                    glo = nc.gpsimd.dma_gather(
                        out_ap=g[:, 0:nlo, :], in_ap=h_full[l][0:GA, :],
                        idxs_ap=srci[:, t0 * 8:t0 * 8 + nlo * 8],
                        num_idxs=nlo * P, num_idxs_reg=nreg(nlo * P), elem_size=TW,
                        single_packet=False, queue_num=nextq())
                    add_dep_helper(glo.ins, ccA[l].ins, sync=True, reason="gather after agA")
                if nlo < CT:
                    ghi = nc.gpsimd.dma_gather(
                        out_ap=g[:, nlo:CT, :], in_ap=h_full[l][GA:npad, :],
                        idxs_ap=srci[:, t0 * 8 + nlo * 8:(t0 + CT) * 8],
                        num_idxs=(CT - nlo) * P, num_idxs_reg=nreg((CT - nlo) * P),
                        elem_size=TW, single_packet=False, queue_num=nextq())
                    add_dep_helper(ghi.ins, ccB[l].ins, sync=True, reason="gather after agB")

                # transposed selectors (node-slot one-hot along partitions)
                dT = sb_t.tile([P, CT, P], mybir.dt.int8, tag="dT")
                nc.sync.dma_start(
                    out=dT[:].rearrange("p c e -> p (c e)"),
                    in_=dlocT_d[0:1, t0 * P:(t0 + CT) * P].to_broadcast(
                        [P, CT * P]))
                selT = sb_t.tile([P, CT, P], bf16, tag="selT")
                nc.vector.tensor_tensor(
                    out=selT[:], in0=dT[:],
                    in1=iotap[:].rearrange("p (c e) -> p c e", c=1).to_broadcast(
                        [P, CT, P]),
                    op=mybir.AluOpType.is_equal)
                # per-edge alpha_dst via PE: adps[:, j, :] = selT_j^T @ adsb[b]
                adps = ps_ad.tile([P, CT, H], f32, tag="adps")
                for t in range(t0, t0 + CT):
                    b = int(blk_of_tile[t])
                    j = int(bufcol[t])
                    nc.tensor.matmul(adps[:, j, :], lhsT=selT[:, j, :],
                                     rhs=adsb[:, b, :], start=True, stop=True)

                # e = lrelu(a_s + a_d); w = exp(e) written into g cols HC:TBL
                lg = sb_w.tile([P, CT, H], f32, tag="lg")
                nc.vector.tensor_tensor(out=lg[:], in0=adps[:],
                                        in1=g[:, :, HC:TBL],
                                        op=mybir.AluOpType.add)
                lr = sb_w.tile([P, CT, H], f32, tag="lr")
                if K_LRELU:
                    nc.scalar.activation(lr[:], lg[:],
                                         mybir.ActivationFunctionType.Lrelu,
                                         alpha=NEG_SLOPE)
                else:
                    nc.vector.tensor_scalar_mul(lr[:], lg[:], NEG_SLOPE)
                    nc.vector.tensor_tensor(out=lr[:], in0=lr[:], in1=lg[:],
                                            op=mybir.AluOpType.max)
                nc.scalar.activation(g[:, :, HC:TBL], lr[:],
                                     mybir.ActivationFunctionType.Exp)
                # msg in place: g[:, :, h*Ch:(h+1)*Ch] *= w[h]
                nc.vector.tensor_tensor(
                    out=g[:, :, 0:HC].rearrange("p c (h w) -> p c h w", h=H),
                    in0=g[:, :, 0:HC].rearrange("p c (h w) -> p c h w", h=H),
                    in1=g[:, :, HC:TBL].rearrange("p c (h o) -> p c h o", o=1)
                        .to_broadcast([P, CT, H, Ch]),
                    op=mybir.AluOpType.mult)
                # selectors for the whole chunk (dst one-hot per edge)
                sel = sb_s.tile([P, CT, P], bf16, tag="sel")
                nc.vector.tensor_tensor(
                    out=sel[:], in0=iotarep[:],
                    in1=dloc[:, t0:t0 + CT, :].to_broadcast([P, CT, P]),
                    op=mybir.AluOpType.is_equal)
                # aggregate per tile (processing order)
                for t in range(t0, t0 + CT):
                    b = int(blk_of_tile[t])
                    j = int(bufcol[t])
                    if start_t[t]:
                        agg_of_blk[b] = ps_agg.tile([P, TBL], f32, tag="agg",
                                                    name=f"agg{l}_{b}")
                    nc.tensor.matmul(agg_of_blk[b][:], lhsT=sel[:, j, :],
                                     rhs=g[:, j, 0:TBL],
                                     start=bool(start_t[t]),
                                     stop=bool(stop_t[t]))
                    if stop_t[t]:
                        agg = agg_of_blk.pop(b)
                        rec = sb_w.tile([P, H], f32, tag="rec")
                        nc.vector.reciprocal(rec[:], agg[:, HC:TBL])
                        xb = sb_w.tile([P, HC], f32, tag="xb")
                        nc.vector.tensor_tensor(
                            out=xb[:].rearrange("p (h w) -> p h w", h=H),
                            in0=agg[:, 0:HC].rearrange("p (h w) -> p h w", h=H),
                            in1=rec[:].rearrange("p (h o) -> p h o", o=1)
                                .to_broadcast([P, H, Ch]),
                            op=mybir.AluOpType.mult)
                        if K_RELU:
                            nc.scalar.activation(xb[:], xb[:],
                                                 mybir.ActivationFunctionType.Relu)
                        else:
                            nc.vector.tensor_scalar_max(xb[:], xb[:], 0.0)
                        if l < 2:
                            xps = ps_xp.tile([P, P], f32, tag="xps")
                            nc.tensor.transpose(xps[:], xb[:], identf[:])
                            nc.vector.tensor_copy(
                                out=xT[:, b * P:(b + 1) * P], in_=xps[:])
                            if b >= 2:
                                node_mm(l + 1, b - 2)
                            if b == nb - 1:
                                node_mm(l + 1, nb - 2)
                                node_mm(l + 1, nb - 1)
                        else:
                            bsel = sb_w.tile([P, GRAPHS], f32, tag="bsel")
                            nc.vector.tensor_tensor(
                                out=bsel[:],
                                in0=batchf[:, b, :].to_broadcast([P, GRAPHS]),
                                in1=iota[:, 0, :GRAPHS],
                                op=mybir.AluOpType.is_equal)
                            nc.tensor.matmul(pooled_ps[:], lhsT=bsel[:],
                                             rhs=xb[:], start=(b == 0),
                                             stop=(b == nb - 1))
            if l < 2:
                if ccA[l + 1] is None:
                    emit_ccA(l + 1)
                emit_ccB(l + 1)

        # ===== head =====
        pooled_sb = sb.tile([GRAPHS, HC], f32)
        nc.vector.tensor_copy(out=pooled_sb[:], in_=pooled_ps[:])
        pT_ps = ps_xp.tile([P, GRAPHS], f32, tag="xps")
        nc.tensor.transpose(pT_ps[:], pooled_sb[:], identf[:GRAPHS, :GRAPHS])
        pT_sb = sb.tile([P, GRAPHS], f32)
        nc.vector.tensor_copy(out=pT_sb[:], in_=pT_ps[:])
        log_ps = ps_xp.tile([GRAPHS, OUT], f32, tag="xps")
        nc.tensor.matmul(log_ps[:], lhsT=pT_sb[:], rhs=wh[:], start=True, stop=True)
        log_sb = sb.tile([GRAPHS, OUT], f32)
        nc.vector.tensor_copy(out=log_sb[:], in_=log_ps[:])
        nc.sync.dma_start(out=out_d[:], in_=log_sb[:])

    _fixup_wait_limits(nc)
    mybir.codegen_inst_isa_subclasses(nc)
    return nc


def prepare(x, Ws, a_srcs, a_dsts, biases, Wh, bh, edge_index, batch):
    n = x.shape[0]
    npad = int(math.ceil(n / (NCORES * P)) * NCORES * P)
    per = npad // NCORES
    nb = per // P

    x = np.asarray(x, np.float32)
    Ws = [np.asarray(w, np.float32) for w in Ws]
    a_srcs = [np.asarray(a, np.float32) for a in a_srcs]
    a_dsts = [np.asarray(a, np.float32) for a in a_dsts]
    Wh = np.asarray(Wh, np.float32)
    bh = np.asarray(bh, np.float32)
    edge_index = np.asarray(edge_index)
    batch = np.asarray(batch)
    for b in biases:
        assert np.allclose(np.asarray(b), 0.0), "nonzero GAT biases unsupported"

    import ml_dtypes
    # W_aug = [W | W@As | W@Ad]
    waugs = []
    for l in range(3):
        As = np.zeros((HC, H), np.float32)
        Ad = np.zeros((HC, H), np.float32)
        for h in range(H):
            As[h * Ch:(h + 1) * Ch, h] = a_srcs[l][h]
            Ad[h * Ch:(h + 1) * Ch, h] = a_dsts[l][h]
        W = Ws[l]
        waugs.append(np.concatenate([W, W @ As, W @ Ad], axis=1))
    waug = np.stack(waugs, 0).astype(ml_dtypes.bfloat16)  # [3, 128, AUGW]

    # edges + self loops (incl. pad nodes, so every row has >=1 edge)
    src_all = np.concatenate([edge_index[0].astype(np.int64),
                              np.arange(npad, dtype=np.int64)])
    dst_all = np.concatenate([edge_index[1].astype(np.int64),
                              np.arange(npad, dtype=np.int64)])
    (blk_of_tile, start_t, stop_t, bufcol, n_lo, Tpad,
     srcidxs, dlocs, dlocTs) = _prep_edges(src_all, dst_all, per, nb, npad)

    xpad = np.zeros((npad, HC), np.float32)
    xpad[:n] = x
    iota = np.tile(np.arange(P, dtype=np.float32)[None, :], (P, 4))

    batchf_full = np.full(npad, -1.0, np.float32)
    batchf_full[:n] = batch.astype(np.float32)

    nc = _build(npad, Tpad, blk_of_tile, start_t, stop_t, bufcol, n_lo)

    in_maps = []
    for c in range(NCORES):
        sl = slice(c * per, (c + 1) * per)
        in_maps.append({
            "xT": np.ascontiguousarray(xpad[sl].T).astype(ml_dtypes.bfloat16),
            "waug": waug,
            "wh": Wh,
            "iota": iota,
            "iotap": np.arange(P, dtype=np.int8)[:, None],
            "ident": np.eye(P, dtype=np.float32),
            "srcidx": srcidxs[c],
            "dloc": dlocs[c],
            "iotarep": np.tile(np.arange(P, dtype=np.float32)[None, :],
                               (P, CT)).astype(ml_dtypes.bfloat16),
            "dlocT": dlocTs[c],
            "batchf": np.ascontiguousarray(
                batchf_full[sl].reshape(nb, P).T),
            })
    return nc, in_maps


def run_gat(x, Ws, a_srcs, a_dsts, biases, Wh, bh, edge_index, batch):
    nc, in_maps = prepare(x, Ws, a_srcs, a_dsts, biases, Wh, bh,
                          edge_index, batch)
    res = run_bass_kernel_spmd(nc, in_maps, list(range(NCORES)))
    global LAST_EXEC_NS
    LAST_EXEC_NS = getattr(res, "exec_time_ns", None)
    logits = np.zeros((GRAPHS, OUT), np.float32)
    for c in range(NCORES):
        logits += res.results[c]["out"]
    return logits + bh


def kernel(**inputs):
    return np.asarray(run_gat(
        inputs["x"], inputs["Ws"], inputs["a_srcs"], inputs["a_dsts"],
        inputs["biases"], inputs["Wh"], inputs["bh"], inputs["edge_index"],
        inputs["batch"]), np.float32)


# revision 29
# speedup vs baseline: 1.1370x; 1.0254x over previous
"""Trainium2 Bass kernel for 3-layer GAT + global_add_pool + linear head.

Design (v3):
- Nodes (and incoming edges) sharded across 8 cores by dst.
- Node phase per layer: h_aug = x @ [W | W@As | W@Ad] on PE (bf16 in, f32 out).
  [h|alpha_src] rows (bf16, 512B stride) go to a DRAM table that is
  AllGathered across cores.
- Edge phase: edges tiled 128/dst-block (sub-split by src half for int16
  gather indices); per chunk of CT tiles, batched InstDMAGatherAnt fetches
  h[src] rows on 4 SWDGE queues. Per-edge alpha_dst comes from transposed
  selector matmuls on the (otherwise idle) PE; selectors are built by vector
  is_equal against a streamed dst-slot pattern.
- e = lrelu(a_s+a_d) on vector (max(z,.2z)), exp batched on scalar, messages
  scaled in place (bf16), then per-tile selector matmuls accumulate [w*h | w]
  into PSUM per dst block; normalization + relu on block end.
- Partial pooled logits summed on host.

Self-contained: no file reads; shapes hardcoded via constants.
"""
import math
import numpy as np
from contextlib import ExitStack

import concourse.bass as bass
import concourse.mybir as mybir
import concourse.tile as tile
from concourse.bass_utils import run_bass_kernel_spmd
from concourse.tile_rust import add_dep_helper
from concourse import library_config

NCORES = 8
P = 128
H = 4
Ch = 32
HC = 128
AUGW = HC + 2 * H   # 136: node matmul out [h | a_src | a_dst]
TBL = HC + H        # 132: useful row [h | a_src]
TW = 256            # bf16 table row width (512B stride)
NEG_SLOPE = 0.2
GRAPHS = 64
OUT = 10
CT = 32             # tiles (of 128 edges) per chunk
NSWQ = 4            # SWDGE queues
BA = 32             # blocks in table piece A (per core), capped to nb//2


def _ba(nb):
    return max(1, min(BA, nb // 2))
CC_HIDE_CHUNK = 20  # edge-phase chunk index at which next layer's AG-A issues
import os as _os
K_LRELU = _os.environ.get("K_LRELU", "0") == "1"
K_RELU = _os.environ.get("K_RELU", "1") == "1"

# instruction types whose BIR struct cannot carry all Tile-emitted waits
_WAIT_CAPS = {
    "InstDMAGatherAnt": 0,
    "InstDMAScatterAddAnt": 0,
    "InstNoOp": 1,
    "InstDrain": 1,
    "InstCollectiveCompute": 1,
}


def _fixup_wait_limits(nc):
    k = 0
    for fn in nc.m.functions:
        for blk in fn.blocks:
            out = []
            for inst in blk.instructions:
                cap = _WAIT_CAPS.get(type(inst).__name__, 1)
                si = inst.sync_info
                if si is not None:
                    waits = list(si.on_wait)
                    if len(waits) > cap:
                        keep, move = waits[:cap], waits[cap:]
                        for w in move:
                            nop = mybir.InstNoOp(name=f"waitfix_{k}", text_hint="wait_fixup")
                            k += 1
                            nop.engine = inst.engine
                            nop.sync_info = type(si)(on_wait=[w], on_update=[])
                            out.append(nop)
                        inst.sync_info = type(si)(on_wait=list(keep), on_update=list(si.on_update))
                out.append(inst)
            blk.instructions = out
    return k


def _prep_edges(src_all, dst_all, per, nb, npad):
    """Tile edges per core by (dst block, src half), build gather idx arrays.

    Returns (blk_of_tile, start_t, stop_t, bufcol, n_lo, Tpad,
    srcidxs, dlocs, dlocTs): srcidx is the per-core [128, Tpad*8] int16 SBUF
    image; dloc is [P, Tpad] f32 dst-slot per edge slot (buffer order, -1
    pad); dlocT is [1, Tpad*128] bf16 dst-slot along free dim (buffer order).
    """
    NHALF = npad // 2
    percore = []
    cnt_bh = np.zeros((nb, 2), np.int64)
    for c in range(NCORES):
        m = (dst_all // per) == c
        s = src_all[m]
        loc = dst_all[m] - c * per
        b = loc // P
        sc = s // per
        srem = s % per
        hf = (srem // P >= _ba(nb)).astype(np.int64)
        order = np.lexsort((hf, b))
        s, loc, b, hf = s[order], loc[order], b[order], hf[order]
        sc, srem = sc[order], srem[order]
        cnt = np.bincount(b * 2 + hf, minlength=nb * 2).reshape(nb, 2)
        cnt_bh = np.maximum(cnt_bh, cnt)
        ba = _ba(nb)
        rloc = np.where(srem < ba * P, sc * (ba * P) + srem,
                        sc * (per - ba * P) + srem - ba * P)
        percore.append((rloc, loc, b, hf, cnt))

    tiles_bh = (cnt_bh + P - 1) // P            # [nb, 2]
    # processing-order tiles
    blk_l, hf_l = [], []
    tstart = np.zeros((nb, 2), np.int64)
    t = 0
    for b in range(nb):
        for hf in (0, 1):
            tstart[b, hf] = t
            n = int(tiles_bh[b, hf])
            blk_l += [b] * n
            hf_l += [hf] * n
            t += n
    T = t
    nchunks = (T + CT - 1) // CT
    Tpad = nchunks * CT
    blk_of_tile = np.array(blk_l + [nb - 1] * (Tpad - T), np.int64)
    hf_of_tile = np.array(hf_l + [0] * (Tpad - T), np.int64)
    start_t = np.zeros(Tpad, bool)
    stop_t = np.zeros(Tpad, bool)
    for b in range(nb):
        w = np.nonzero(blk_of_tile == b)[0]
        start_t[w[0]] = True
        stop_t[w[-1]] = True

    # buffer-column mapping: per chunk, lo tiles first then hi tiles
    bufcol = np.zeros(Tpad, np.int64)
    n_lo = []
    for ch in range(nchunks):
        ts = np.arange(ch * CT, (ch + 1) * CT)
        lo = ts[hf_of_tile[ts] == 0]
        hi = ts[hf_of_tile[ts] == 1]
        bufcol[lo] = np.arange(len(lo))
        bufcol[hi] = len(lo) + np.arange(len(hi))
        n_lo.append(len(lo))
    g2b = (np.arange(Tpad) // CT) * CT + bufcol   # proc tile -> buffer col

    import ml_dtypes
    SW = Tpad * 8
    srcidxs, dlocs, dlocTs = [], [], []
    for c in range(NCORES):
        rloc, loc, b, hf, cnt = percore[c]
        srcv = np.zeros((P, Tpad), np.int64)
        dlp = np.full((P, Tpad), -1.0, np.float32)   # proc order
        off = np.zeros(nb * 2 + 1, np.int64)
        off[1:] = np.cumsum(cnt.reshape(-1))
        key = b * 2 + hf
        pos = np.arange(len(rloc)) - off[key]
        tt = tstart[b, hf] + pos // P
        pp = pos % P
        srcv[pp, tt] = rloc
        dlp[pp, tt] = loc % P
        # sbuf idx image: slot (p, t) -> row 16k+p%16, col (bufgcol*8)+p//16
        p_g, t_g = np.mgrid[0:P, 0:Tpad]
        col = g2b[t_g] * 8 + p_g // 16
        row = p_g % 16
        si = np.zeros((P, SW), np.int16)
        for k in range(8):
            si[16 * k + row, col] = srcv
        srcidxs.append(si)
        dli = dlp.astype(np.int64)                      # [P, Tpad], -1 pad
        ar = np.arange(P)
        selb = (dli[:, :, None] == ar[None, None, :])   # [e, t, n]
        sel_s = np.zeros((P, Tpad, P), np.uint8)
        sel_s[:, g2b, :] = selb
        selTb = np.transpose(selb, (2, 1, 0))           # [n, t, e]
        selT_s = np.zeros((P, Tpad, P), np.uint8)
        selT_s[:, g2b, :] = selTb
        one = np.float32(1.0).astype(ml_dtypes.float8_e4m3).view(np.uint8)
        dlocs.append((sel_s * one).view(ml_dtypes.float8_e4m3)
                     .reshape(P, Tpad * P))
        dlocTs.append((selT_s * one).view(ml_dtypes.float8_e4m3)
                      .reshape(P, Tpad * P))
    return (blk_of_tile, start_t, stop_t, bufcol, n_lo, Tpad,
            srcidxs, dlocs, dlocTs)


def _build(npad, Tpad, blk_of_tile, start_t, stop_t, bufcol, n_lo):
    per = npad // NCORES
    nb = per // P
    NHALF = npad // 2
    nlayers = 3
    nchunks = Tpad // CT
    f32 = mybir.dt.float32
    bf16 = mybir.dt.bfloat16
    i16 = mybir.dt.int16

    nc = bass.Bass(num_devices=NCORES, num_swdge_queues=NSWQ)
    # ---- dram I/O
    xT_d = nc.dram_tensor("xT", [P, per], bf16, kind="ExternalInput")
    waug_d = nc.dram_tensor("waug", [nlayers, P, AUGW], bf16, kind="ExternalInput")
    wh_d = nc.dram_tensor("wh", [P, OUT], f32, kind="ExternalInput")
    iota_d = nc.dram_tensor("iota", [P, 4 * P], f32, kind="ExternalInput")
    ident_d = nc.dram_tensor("ident", [P, P], f32, kind="ExternalInput")
    SW = Tpad * 8
    srcidx_d = nc.dram_tensor("srcidx", [P, SW], i16, kind="ExternalInput")
    sels_d = nc.dram_tensor("sels", [P, Tpad * P], mybir.dt.float8e4,
                            kind="ExternalInput")
    selTs_d = nc.dram_tensor("selTs", [P, Tpad * P], mybir.dt.float8e4,
                             kind="ExternalInput")
    batchf_d = nc.dram_tensor("batchf", [P, nb], f32, kind="ExternalInput")
    out_d = nc.dram_tensor("out", [GRAPHS, OUT], f32, kind="ExternalOutput")

    h_loc = [nc.dram_tensor(f"h_loc{l}", [per, TW], bf16) for l in range(nlayers)]
    h_full = [nc.dram_tensor(f"h_full{l}", [npad, TW], bf16, addr_space="Shared")
              for l in range(nlayers)]

    groups = [list(range(NCORES))]

    with ExitStack() as ctx:
        tc = ctx.enter_context(tile.TileContext(nc))
        sb = ctx.enter_context(tc.tile_pool(name="sb", bufs=1))
        sb_g = ctx.enter_context(tc.tile_pool(name="sbg", bufs=3))
        sb_s = ctx.enter_context(tc.tile_pool(name="sbs", bufs=3))
        sb_t = ctx.enter_context(tc.tile_pool(name="sbt", bufs=3))
        sb_w = ctx.enter_context(tc.tile_pool(name="sbw", bufs=3))
        ps_h = ctx.enter_context(tc.tile_pool(name="psh", bufs=2, space="PSUM"))
        ps_agg = ctx.enter_context(tc.tile_pool(name="psagg", bufs=2, space="PSUM"))
        ps_ad = ctx.enter_context(tc.tile_pool(name="psad", bufs=2, space="PSUM"))
        ps_xp = ctx.enter_context(tc.tile_pool(name="psxp", bufs=1, space="PSUM"))
        ps_fin = ctx.enter_context(tc.tile_pool(name="psfin", bufs=1, space="PSUM"))

        # ---- persistent SBUF state
        xT = sb.tile([P, per], bf16)
        nc.sync.dma_start(out=xT[:], in_=xT_d[:])
        waug = sb.tile([P, nlayers, AUGW], bf16)
        nc.sync.dma_start(out=waug[:],
                          in_=waug_d[:].rearrange("l p a -> p l a"))
        wh = sb.tile([P, OUT], f32)
        nc.sync.dma_start(out=wh[:], in_=wh_d[:])
        iota = sb.tile([P, 4, P], f32)
        nc.sync.dma_start(out=iota[:].rearrange("p a b -> p (a b)"), in_=iota_d[:])
        srci = sb.tile([P, SW], i16)
        nc.sync.dma_start(out=srci[:], in_=srcidx_d[:])
        batchf = sb.tile([P, nb, 1], f32)
        nc.sync.dma_start(out=batchf[:].rearrange("p b o -> p (b o)"), in_=batchf_d[:])
        identf = sb.tile([P, P], f32)
        nc.sync.dma_start(out=identf[:], in_=ident_d[:])

        nc.gpsimd.load_library(library_config.mlp)

        _regs = {}

        def nreg(v):
            if v not in _regs:
                _regs[v] = nc.gpsimd.to_reg(v)
            return _regs[v]

        hsb2 = [sb.tile([P, nb, TBL], bf16, name=f"hsb{i}") for i in range(2)]
        adsb2 = [sb.tile([P, nb, H], bf16, name=f"adsb{i}") for i in range(2)]
        pooled_ps = ps_fin.tile([GRAPHS, HC], f32)
        qn = [0]

        def nextq():
            qn[0] = (qn[0] + 1) % NSWQ
            return qn[0]

        ba = _ba(nb)
        RA = ba * P            # piece-A local rows
        GA = NCORES * RA       # piece-A global rows
        dhA = [None] * 3
        dhB = [None] * 3
        ccA = [None] * 3
        ccB = [None] * 3

        def node_mm(l, b):
            hsb, adsb = hsb2[l % 2], adsb2[l % 2]
            ps = ps_h.tile([P, AUGW], f32, tag="ndps")
            nc.tensor.matmul(ps[:], lhsT=xT[:, b * P:(b + 1) * P],
                             rhs=waug[:, l, :], start=True, stop=True)
            nc.vector.tensor_copy(out=hsb[:, b, :], in_=ps[:, :TBL])
            nc.vector.tensor_copy(out=adsb[:, b, :], in_=ps[:, TBL:AUGW])
            if b == ba - 1:
                dhA[l] = nc.sync.dma_start(
                    out=h_loc[l][0:RA, 0:TBL].rearrange("(b p) d -> p b d", p=P),
                    in_=hsb[:, 0:ba, :])
            if b == nb - 1:
                dhB[l] = nc.sync.dma_start(
                    out=h_loc[l][RA:per, 0:TBL].rearrange("(b p) d -> p b d", p=P),
                    in_=hsb[:, ba:nb, :])

        def emit_ccA(l):
            ccA[l] = nc.gpsimd.collective_compute(
                "AllGather", mybir.AluOpType.bypass, replica_groups=groups,
                ins=[h_loc[l][0:RA, :]], outs=[h_full[l][0:GA, :]])
            add_dep_helper(ccA[l].ins, dhA[l].ins, sync=True, reason="hA before ag")

        def emit_ccB(l):
            ccB[l] = nc.gpsimd.collective_compute(
                "AllGather", mybir.AluOpType.bypass, replica_groups=groups,
                ins=[h_loc[l][RA:per, :]], outs=[h_full[l][GA:npad, :]])
            add_dep_helper(ccB[l].ins, dhB[l].ins, sync=True, reason="hB before ag")

        # layer-0 node phase prologue
        for b in range(nb):
            node_mm(0, b)
        emit_ccA(0)
        emit_ccB(0)

        for l in range(3):
            adsb = adsb2[l % 2]
            # ===== edge phase =====
            agg_of_blk = {}
            for ci in range(nchunks):
                t0 = ci * CT
                nlo = int(n_lo[ci])
                if (l < 2 and ccA[l + 1] is None and dhA[l + 1] is not None
                        and ci >= CC_HIDE_CHUNK):
                    emit_ccA(l + 1)
                g = sb_g.tile([P, CT, TW], bf16, tag="gath")
                if nlo > 0:
                    glo = nc.gpsimd.dma_gather(
                        out_ap=g[:, 0:nlo, :], in_ap=h_full[l][0:GA, :],
                        idxs_ap=srci[:, t0 * 8:t0 * 8 + nlo * 8],
                        num_idxs=nlo * P, num_idxs_reg=nreg(nlo * P), elem_size=TW,
                        single_packet=False, queue_num=nextq())
                    add_dep_helper(glo.ins, ccA[l].ins, sync=True, reason="gather after agA")
                if nlo < CT:
                    ghi = nc.gpsimd.dma_gather(
                        out_ap=g[:, nlo:CT, :], in_ap=h_full[l][GA:npad, :],
                        idxs_ap=srci[:, t0 * 8 + nlo * 8:(t0 + CT) * 8],
                        num_idxs=(CT - nlo) * P, num_idxs_reg=nreg((CT - nlo) * P),
                        elem_size=TW, single_packet=False, queue_num=nextq())
                    add_dep_helper(ghi.ins, ccB[l].ins, sync=True, reason="gather after agB")

                # transposed selectors (node-slot one-hot along partitions)
                selT = sb_t.tile([P, CT, P], mybir.dt.float8e4, tag="selT")
                nc.sync.dma_start(
                    out=selT[:].rearrange("p c e -> p (c e)"),
                    in_=selTs_d[:, t0 * P:(t0 + CT) * P])
                # per-edge alpha_dst via PE: adps[:, j, :] = selT_j^T @ adsb[b]
                adps = ps_ad.tile([P, CT, H], f32, tag="adps")
                for t in range(t0, t0 + CT):
                    b = int(blk_of_tile[t])
                    j = int(bufcol[t])
                    nc.tensor.matmul(adps[:, j, :], lhsT=selT[:, j, :],
                                     rhs=adsb[:, b, :], start=True, stop=True)

                # e = lrelu(a_s + a_d); w = exp(e) written into g cols HC:TBL
                lg = sb_w.tile([P, CT, H], f32, tag="lg")
                nc.vector.tensor_tensor(out=lg[:], in0=adps[:],
                                        in1=g[:, :, HC:TBL],
                                        op=mybir.AluOpType.add)
                lr = sb_w.tile([P, CT, H], f32, tag="lr")
                if K_LRELU:
                    nc.scalar.activation(lr[:], lg[:],
                                         mybir.ActivationFunctionType.Lrelu,
                                         alpha=NEG_SLOPE)
                else:
                    nc.vector.tensor_scalar_mul(lr[:], lg[:], NEG_SLOPE)
                    nc.vector.tensor_tensor(out=lr[:], in0=lr[:], in1=lg[:],
                                            op=mybir.AluOpType.max)
                nc.scalar.activation(g[:, :, HC:TBL], lr[:],
                                     mybir.ActivationFunctionType.Exp)
                # msg in place: g[:, :, h*Ch:(h+1)*Ch] *= w[h]
                nc.vector.tensor_tensor(
                    out=g[:, :, 0:HC].rearrange("p c (h w) -> p c h w", h=H),
                    in0=g[:, :, 0:HC].rearrange("p c (h w) -> p c h w", h=H),
                    in1=g[:, :, HC:TBL].rearrange("p c (h o) -> p c h o", o=1)
                        .to_broadcast([P, CT, H, Ch]),
                    op=mybir.AluOpType.mult)
                # selectors for the whole chunk (dst one-hot per edge)
                sel = sb_s.tile([P, CT, P], mybir.dt.float8e4, tag="sel")
                nc.sync.dma_start(
                    out=sel[:].rearrange("p c e -> p (c e)"),
                    in_=sels_d[:, t0 * P:(t0 + CT) * P])
                # aggregate per tile (processing order)
                for t in range(t0, t0 + CT):
                    b = int(blk_of_tile[t])
                    j = int(bufcol[t])
                    if start_t[t]:
                        agg_of_blk[b] = ps_agg.tile([P, TBL], f32, tag="agg",
                                                    name=f"agg{l}_{b}")
                    nc.tensor.matmul(agg_of_blk[b][:], lhsT=sel[:, j, :],
                                     rhs=g[:, j, 0:TBL],
                                     start=bool(start_t[t]),
                                     stop=bool(stop_t[t]))
                    if stop_t[t]:
                        agg = agg_of_blk.pop(b)
                        rec = sb_w.tile([P, H], f32, tag="rec")
                        nc.vector.reciprocal(rec[:], agg[:, HC:TBL])
                        xb = sb_w.tile([P, HC], f32, tag="xb")
                        nc.vector.tensor_tensor(
                            out=xb[:].rearrange("p (h w) -> p h w", h=H),
                            in0=agg[:, 0:HC].rearrange("p (h w) -> p h w", h=H),
                            in1=rec[:].rearrange("p (h o) -> p h o", o=1)
                                .to_broadcast([P, H, Ch]),
                            op=mybir.AluOpType.mult)
                        if K_RELU:
                            nc.scalar.activation(xb[:], xb[:],
                                                 mybir.ActivationFunctionType.Relu)
                        else:
                            nc.vector.tensor_scalar_max(xb[:], xb[:], 0.0)
                        if l < 2:
                            xps = ps_xp.tile([P, P], f32, tag="xps")
                            nc.tensor.transpose(xps[:], xb[:], identf[:])
                            nc.vector.tensor_copy(
                                out=xT[:, b * P:(b + 1) * P], in_=xps[:])
                            if b >= 2:
                                node_mm(l + 1, b - 2)
                            if b == nb - 1:
                                node_mm(l + 1, nb - 2)
                                node_mm(l + 1, nb - 1)
                        else:
                            bsel = sb_w.tile([P, GRAPHS], f32, tag="bsel")
                            nc.vector.tensor_tensor(
                                out=bsel[:],
                                in0=batchf[:, b, :].to_broadcast([P, GRAPHS]),
                                in1=iota[:, 0, :GRAPHS],
                                op=mybir.AluOpType.is_equal)
                            nc.tensor.matmul(pooled_ps[:], lhsT=bsel[:],
                                             rhs=xb[:], start=(b == 0),
                                             stop=(b == nb - 1))
            if l < 2:
                if ccA[l + 1] is None:
                    emit_ccA(l + 1)
                emit_ccB(l + 1)

        # ===== head =====
        pooled_sb = sb.tile([GRAPHS, HC], f32)
        nc.vector.tensor_copy(out=pooled_sb[:], in_=pooled_ps[:])
        pT_ps = ps_xp.tile([P, GRAPHS], f32, tag="xps")
        nc.tensor.transpose(pT_ps[:], pooled_sb[:], identf[:GRAPHS, :GRAPHS])
        pT_sb = sb.tile([P, GRAPHS], f32)
        nc.vector.tensor_copy(out=pT_sb[:], in_=pT_ps[:])
        log_ps = ps_xp.tile([GRAPHS, OUT], f32, tag="xps")
        nc.tensor.matmul(log_ps[:], lhsT=pT_sb[:], rhs=wh[:], start=True, stop=True)
        log_sb = sb.tile([GRAPHS, OUT], f32)
        nc.vector.tensor_copy(out=log_sb[:], in_=log_ps[:])
        nc.sync.dma_start(out=out_d[:], in_=log_sb[:])

    _fixup_wait_limits(nc)
    mybir.codegen_inst_isa_subclasses(nc)
    return nc


def prepare(x, Ws, a_srcs, a_dsts, biases, Wh, bh, edge_index, batch):
    n = x.shape[0]
    npad = int(math.ceil(n / (NCORES * P)) * NCORES * P)
    per = npad // NCORES
    nb = per // P

    x = np.asarray(x, np.float32)
    Ws = [np.asarray(w, np.float32) for w in Ws]
    a_srcs = [np.asarray(a, np.float32) for a in a_srcs]
    a_dsts = [np.asarray(a, np.float32) for a in a_dsts]
    Wh = np.asarray(Wh, np.float32)
    bh = np.asarray(bh, np.float32)
    edge_index = np.asarray(edge_index)
    batch = np.asarray(batch)
    for b in biases:
        assert np.allclose(np.asarray(b), 0.0), "nonzero GAT biases unsupported"

    import ml_dtypes
    # W_aug = [W | W@As | W@Ad]
    waugs = []
    for l in range(3):
        As = np.zeros((HC, H), np.float32)
        Ad = np.zeros((HC, H), np.float32)
        for h in range(H):
            As[h * Ch:(h + 1) * Ch, h] = a_srcs[l][h]
            Ad[h * Ch:(h + 1) * Ch, h] = a_dsts[l][h]
        W = Ws[l]
        waugs.append(np.concatenate([W, W @ As, W @ Ad], axis=1))
    waug = np.stack(waugs, 0).astype(ml_dtypes.bfloat16)  # [3, 128, AUGW]

    # edges + self loops (incl. pad nodes, so every row has >=1 edge)
    src_all = np.concatenate([edge_index[0].astype(np.int64),
                              np.arange(npad, dtype=np.int64)])
    dst_all = np.concatenate([edge_index[1].astype(np.int64),
                              np.arange(npad, dtype=np.int64)])
    (blk_of_tile, start_t, stop_t, bufcol, n_lo, Tpad,
     srcidxs, dlocs, dlocTs) = _prep_edges(src_all, dst_all, per, nb, npad)

    xpad = np.zeros((npad, HC), np.float32)
    xpad[:n] = x
    iota = np.tile(np.arange(P, dtype=np.float32)[None, :], (P, 4))

    batchf_full = np.full(npad, -1.0, np.float32)
    batchf_full[:n] = batch.astype(np.float32)

    nc = _build(npad, Tpad, blk_of_tile, start_t, stop_t, bufcol, n_lo)

    in_maps = []
    for c in range(NCORES):
        sl = slice(c * per, (c + 1) * per)
        in_maps.append({
            "xT": np.ascontiguousarray(xpad[sl].T).astype(ml_dtypes.bfloat16),
            "waug": waug,
            "wh": Wh,
            "iota": iota,
            "ident": np.eye(P, dtype=np.float32),
            "srcidx": srcidxs[c],
            "sels": dlocs[c],
            "selTs": dlocTs[c],
            "batchf": np.ascontiguousarray(
                batchf_full[sl].reshape(nb, P).T),
            })
    return nc, in_maps


def run_gat(x, Ws, a_srcs, a_dsts, biases, Wh, bh, edge_index, batch):
    nc, in_maps = prepare(x, Ws, a_srcs, a_dsts, biases, Wh, bh,
                          edge_index, batch)
    res = run_bass_kernel_spmd(nc, in_maps, list(range(NCORES)))
    global LAST_EXEC_NS
    LAST_EXEC_NS = getattr(res, "exec_time_ns", None)
    logits = np.zeros((GRAPHS, OUT), np.float32)
    for c in range(NCORES):
        logits += res.results[c]["out"]
    return logits + bh


def kernel(**inputs):
    return np.asarray(run_gat(
        inputs["x"], inputs["Ws"], inputs["a_srcs"], inputs["a_dsts"],
        inputs["biases"], inputs["Wh"], inputs["bh"], inputs["edge_index"],
        inputs["batch"]), np.float32)
